# revision 1
# baseline (speedup 1.0000x reference)
"""CascadeGDCN (3-hop graph diffusion convolution) on 8 Trainium2 NeuronCores.

Strategy (matches the sharding hint):
  - Destination nodes sharded across the 8 cores (12544 rows each, padded to
    100352 total).  Edges partitioned by destination core.
  - The full feature matrix X is replicated in every core's DRAM; after each
    hop an AllGather rebuilds it from the per-core output shards.
  - Per SpMM: per-core edges are bucketed into cells (128-dest "group" x
    25088-row source "chunk"; the chunk split keeps dma_gather's int16 source
    indices in range).  Messages are fetched with dma_gather (256B rows), the
    segment reduction runs on the TensorEngine as S^T @ M where S is a
    [128 edges x 128 dests] one-hot-times-val matrix built on-chip by two
    batched VectorEngine ops (iota compare + multiply).  PSUM accumulates the
    20 tiles (4 chunks x 5 slots) of each dest group.
  - Final stage: sum_term^T via PE transpose, z = Theta^T @ st_fm on PE,
    sigmoid on ScalarE, + H on VectorE, output written feature-major and
    transposed back on the host.
"""

import numpy as np

D = 64
NCORES = 8
NUM_HOPS = 3
N_NODES = 100000
SHARD = 12544            # dest rows per core (98 groups of 128)
NODES_PAD = SHARD * NCORES   # 100352
NCHUNKS = 4
CHUNK = NODES_PAD // NCHUNKS  # 25088 (< 32768 so chunk-local idx fits int16)
GROUPS = SHARD // 128    # 98
GPB = 4                  # dest groups per block (per gather call)
MIN_CAP = 5              # min slots (128-edge tiles) per (group, chunk) cell
SKIP_FINAL = False       # dev bisect flag
SKIP_SPMM = False        # dev bisect flag


def _softmax(x):
    e = np.exp(x - x.max())
    return e / e.sum()


def _blocks(groups, gpb):
    out = []
    g = 0
    while g < groups:
        out.append((g, min(gpb, groups - g)))
        g += gpb
    return out


def _layout(groups, gpb, cap):
    """Edge-stream layout: [block][chunk][group_in_block][cap*128]."""
    cap_e = cap * 128
    blocks = _blocks(groups, gpb)
    block_base = []
    base = 0
    for _, gc in blocks:
        block_base.append(base)
        base += NCHUNKS * gc * cap_e
    return blocks, block_base, base  # base == total edge slots


def _prep_direction(dest, src, val, shard, groups, gpb, cap, chunk):
    """Build per-core gather/S tables for one SpMM direction.

    Returns list (per core) of dicts with idx/denc/val device tables.
    """
    cap_e = cap * 128
    blocks, block_base, tot = _layout(groups, gpb, cap)
    ncells = groups * NCHUNKS

    # cell base offset for (g, c)
    cell_base = np.empty(ncells, dtype=np.int64)
    for bi, (g0, gc) in enumerate(blocks):
        for gl in range(gc):
            for c in range(NCHUNKS):
                cell_base[(g0 + gl) * NCHUNKS + c] = (
                    block_base[bi] + c * gc * cap_e + gl * cap_e
                )

    core = dest // shard
    out = []
    for m in range(NCORES):
        sel = core == m
        d_loc = (dest[sel] - m * shard).astype(np.int64)
        s = src[sel].astype(np.int64)
        v = val[sel].astype(np.float32)
        g = d_loc >> 7
        c = s // chunk
        cell = g * NCHUNKS + c
        order = np.argsort(cell, kind="stable")
        cell_s = cell[order]
        counts = np.bincount(cell_s, minlength=ncells)
        if counts.max() > cap_e:
            raise OverflowError(int(np.ceil(counts.max() / 128)))
        starts = np.zeros(ncells, dtype=np.int64)
        starts[1:] = np.cumsum(counts)[:-1]
        rank = np.arange(cell_s.size) - starts[cell_s]
        pos = cell_base[cell_s] + rank

        idx_st = np.zeros(tot, dtype=np.int16)
        denc_st = np.full(tot, -1.0, dtype=np.float32)
        val_st = np.zeros(tot, dtype=np.float32)
        idx_st[pos] = (s[order] - c[order] * chunk).astype(np.int16)
        denc_st[pos] = (d_loc[order] & 127).astype(np.float32)
        val_st[pos] = v[order]

        # the 16-row wrapped block must be replicated into all 8 Q7-core
        # stripes (hardware reads its own 16-partition stripe)
        idx_tbl = np.tile(np.ascontiguousarray(idx_st.reshape(-1, 16).T),
                          (8, 1))
        denc_tbl = np.ascontiguousarray(denc_st.reshape(-1, 128).T)
        val_tbl = np.ascontiguousarray(val_st.reshape(-1, 128).T)
        out.append({"idx": idx_tbl, "denc": denc_tbl, "val": val_tbl})
    return out


def prep_host(H_l, edge_row, edge_col, edge_val, out_degree, in_degree,
              hop_attention, theta_out, theta_in, Theta,
              n_nodes=N_NODES, shard=SHARD, groups=GROUPS, gpb=GPB,
              chunk=CHUNK, min_cap=MIN_CAP):
    """Host-side preprocessing: per-core input maps + meta for the builder."""
    nodes_pad = shard * NCORES
    H = np.asarray(H_l, dtype=np.float32)
    er = np.asarray(edge_row, dtype=np.int64)
    ec = np.asarray(edge_col, dtype=np.int64)
    ev = np.asarray(edge_val, dtype=np.float32)
    od = np.asarray(out_degree, dtype=np.float32)
    idg = np.asarray(in_degree, dtype=np.float32)

    alpha = _softmax(np.asarray(hop_attention, dtype=np.float64))
    th_o = np.asarray(theta_out, dtype=np.float64)
    th_i = np.asarray(theta_in, dtype=np.float64)
    coef = [(float(alpha[k] * th_o[k]), float(alpha[k] * th_i[k]))
            for k in range(len(alpha))]

    cap = min_cap
    while True:
        try:
            # dir 0 ("out" chain): dest=row, src=col; dir 1: transposed
            t0 = _prep_direction(er, ec, ev, shard, groups, gpb, cap, chunk)
            t1 = _prep_direction(ec, er, ev, shard, groups, gpb, cap, chunk)
            break
        except OverflowError as e:
            cap = max(cap + 1, int(e.args[0]))

    def _tf32(x):
        i = np.asarray(x, np.float32).view(np.int32)
        return ((i + 0x1000) & ~0x1FFF).astype(np.int32).view(np.float32)

    x0o = np.zeros((nodes_pad, D), dtype=np.float32)
    x0i = np.zeros((nodes_pad, D), dtype=np.float32)
    x0o[:n_nodes] = _tf32(np.maximum(od, 1e-8)[:, None] * H)
    x0i[:n_nodes] = _tf32(np.maximum(idg, 1e-8)[:, None] * H)

    hpad = np.zeros((nodes_pad, D), dtype=np.float32)
    hpad[:n_nodes] = H
    ident = np.eye(128, dtype=np.float32)
    theta = np.ascontiguousarray(np.asarray(Theta, dtype=np.float32))

    in_maps = []
    for m in range(NCORES):
        in_maps.append({
            "x0_out": x0o,
            "x0_in": x0i,
            "hfm": np.ascontiguousarray(hpad[m * shard:(m + 1) * shard].T),
            "theta": theta,
            "ident": ident,
            "idx0": t0[m]["idx"], "denc0": t0[m]["denc"], "val0": t0[m]["val"],
            "idx1": t1[m]["idx"], "denc1": t1[m]["denc"], "val1": t1[m]["val"],
        })
    meta = {"cap": cap, "coef": coef, "shard": shard, "groups": groups,
            "gpb": gpb, "chunk": chunk, "nodes_pad": nodes_pad}
    return in_maps, meta


def build_program(tc, ins, outs, meta):
    """Emit the full SPMD program into TileContext tc.

    ins/outs: dicts of bass APs (DRAM), keys as in prep_host in_maps + "y".
    """
    import concourse.mybir as mybir

    nc = tc.nc
    f32 = mybir.dt.float32
    f32r = mybir.dt.float32r
    i16 = mybir.dt.int16
    EQ, MUL, ADD = (mybir.AluOpType.is_equal, mybir.AluOpType.mult,
                    mybir.AluOpType.add)

    cap = meta["cap"]
    coef = meta["coef"]
    shard = meta["shard"]
    groups = meta["groups"]
    gpb = meta["gpb"]
    chunk = meta["chunk"]
    nodes_pad = meta["nodes_pad"]
    cap_e = cap * 128
    blocks, block_base, tot = _layout(groups, gpb, cap)
    nslots_tot = tot // 128
    rg = [list(range(NCORES))]

    # internal DRAM: per-direction bounce shard + ping-pong full buffers
    bounce = [nc.dram_tensor(f"bounce{d}", [shard, D], f32r,
                            kind="Internal") for d in range(2)]
    xbuf = [[nc.dram_tensor(f"xbuf{d}_{p}", [nodes_pad, D], f32r,
                            kind="Internal", addr_space="Shared")
             for p in range(2)] for d in range(2)]

    tabs = [
        (ins["idx0"], ins["denc0"], ins["val0"]),
        (ins["idx1"], ins["denc1"], ins["val1"]),
    ]
    x0 = [ins["x0_out"], ins["x0_in"]]

    with (
        tc.tile_pool(name="const", bufs=1) as cpool,
        tc.tile_pool(name="work", bufs=1) as wpool,
        tc.tile_pool(name="stream", bufs=4) as spool,
        tc.tile_pool(name="spool2", bufs=4) as spool2,
        tc.tile_pool(name="fin", bufs=2) as fpool,
        tc.tile_pool(name="ps", bufs=4, space="PSUM") as pspool,
        tc.tile_pool(name="psf", bufs=2, space="PSUM") as psfpool,
    ):
        iota = cpool.tile([128, gpb * cap * 128], f32, tag="iota")
        nc.gpsimd.iota(iota[:], pattern=[[0, gpb * cap], [1, 128]], base=0,
                       channel_multiplier=0,
                       allow_small_or_imprecise_dtypes=True)
        ident_s = cpool.tile([128, 128], f32, tag="ident")
        nc.sync.dma_start(ident_s[:], ins["ident"][:])
        theta_s = cpool.tile([64, D], f32, tag="theta")
        nc.sync.dma_start(theta_s[:], ins["theta"][:])

        st = wpool.tile([128, groups, D], f32, tag="st")
        nc.vector.memset(st[:], 0.0)

        for hop in range(0 if SKIP_SPMM else NUM_HOPS):
            for dirn in range(2):
                idx_d, denc_d, val_d = tabs[dirn]
                xsrc = x0[dirn] if hop == 0 else xbuf[dirn][(hop - 1) % 2]

                denc_s = wpool.tile([128, nslots_tot], f32,
                                    tag=f"denc{dirn}")
                val_s = wpool.tile([128, nslots_tot], f32, tag=f"val{dirn}")
                nc.sync.dma_start(denc_s[:], denc_d[:])
                nc.sync.dma_start(val_s[:], val_d[:])

                xnew = wpool.tile([128, groups, D], f32r, tag=f"xnew{dirn}")

                for bi, (g0, gc) in enumerate(blocks):
                    L = gc * cap_e           # idxs per call
                    ns = gc * cap            # slots per call
                    per_call = []
                    for c in range(NCHUNKS):
                        eoff = block_base[bi] + c * L
                        soff = eoff // 128
                        idx_t = spool.tile([128, L // 16], i16, tag="idx")
                        nc.sync.dma_start(
                            idx_t[:], idx_d[:, eoff // 16:(eoff + L) // 16])
                        msgs = spool.tile([128, ns, D], f32r, tag="msgs")
                        nc.gpsimd.dma_gather(
                            out_ap=msgs[:],
                            in_ap=xsrc[c * chunk:(c + 1) * chunk, :].bitcast(
                                f32r),
                            idxs_ap=idx_t[:],
                            num_idxs=L,
                            num_idxs_reg=L,
                            elem_size=D,
                            single_packet=False,
                            queue_num=c,
                        )
                        S = spool2.tile([128, ns, 128], f32r, tag="S")
                        iota_v = iota[:].rearrange(
                            "p (s c) -> p s c", c=128)[:, :ns, :]
                        nc.vector.tensor_tensor(
                            out=S[:], in0=iota_v,
                            in1=denc_s[:, soff:soff + ns].broadcast_to(
                                [128, ns, 128]),
                            op=EQ)
                        nc.vector.tensor_tensor(
                            out=S[:], in0=S[:].bitcast(f32),
                            in1=val_s[:, soff:soff + ns].broadcast_to(
                                [128, ns, 128]),
                            op=MUL)
                        per_call.append((msgs, S))

                    for gl in range(gc):
                        g = g0 + gl
                        ps = pspool.tile([128, D], f32, tag="ps")
                        for c in range(NCHUNKS):
                            msgs, S = per_call[c]
                            for s in range(cap):
                                sl = gl * cap + s
                                nc.tensor.matmul(
                                    ps[:],
                                    lhsT=S[:, sl, :],
                                    rhs=msgs[:, sl, :],
                                    start=(c == 0 and s == 0),
                                    stop=(c == NCHUNKS - 1 and s == cap - 1),
                                )
                        nc.scalar.copy(out=xnew[:, g, :], in_=ps[:])
                        nc.vector.scalar_tensor_tensor(
                            out=st[:, g, :], in0=ps[:],
                            scalar=coef[hop][dirn], in1=st[:, g, :],
                            op0=MUL, op1=ADD)

                bounce_v = bounce[dirn].ap().rearrange(
                    "(g p) f -> p g f", p=128)
                nc.sync.dma_start(bounce_v, xnew[:])
                if hop < NUM_HOPS - 1:
                    nc.gpsimd.collective_compute(
                        "AllGather", mybir.AluOpType.bypass,
                        replica_groups=rg,
                        ins=[bounce[dirn].ap().opt()],
                        outs=[xbuf[dirn][hop % 2].ap().opt()],
                    )

        # final: y_fm = sigmoid(Theta^T @ st_fm) + H_fm, feature-major
        if SKIP_FINAL:
            for g in range(groups):
                yv = outs["y"][:, g * 128:(g + 1) * 128].rearrange(
                    "f p -> p f")
                nc.sync.dma_start(yv, st[:, g, :])
            return
        fchunks = [(i * 4, min(4, groups - i * 4))
                   for i in range((groups + 3) // 4)]
        for ci, (gs, gcnt) in enumerate(fchunks):
            width = gcnt * 128
            stfm = fpool.tile([64, width], f32, tag="stfm")
            for j in range(gcnt):
                pt = psfpool.tile([64, 128], f32, tag="pt")
                nc.tensor.transpose(pt[:], st[:, gs + j, :], ident_s[:])
                nc.scalar.copy(out=stfm[:, j * 128:(j + 1) * 128], in_=pt[:])
            zp = psfpool.tile([64, width], f32, tag="zp")
            nc.tensor.matmul(zp[:], lhsT=theta_s[:], rhs=stfm[:],
                             start=True, stop=True)
            sg = fpool.tile([64, width], f32, tag="sg")
            nc.scalar.activation(sg[:], zp[:],
                                 mybir.ActivationFunctionType.Sigmoid)
            hf = fpool.tile([64, width], f32, tag="hf")
            nc.sync.dma_start(
                hf[:], ins["hfm"][:, gs * 128:gs * 128 + width])
            yt = fpool.tile([64, width], f32, tag="yt")
            nc.vector.tensor_tensor(out=yt[:], in0=sg[:], in1=hf[:], op=ADD)
            nc.sync.dma_start(
                outs["y"][:, gs * 128:gs * 128 + width], yt[:])


def kernel(**inputs) -> np.ndarray:
    return _run(inputs, trace=False)[0]


def kernel_traced(inputs, trace_kwargs=None):
    """Returns (output, BassKernelResults) with NTFF trace if available."""
    return _run(inputs, trace=True, trace_kwargs=trace_kwargs or {})


def _run(inputs, trace=False, trace_kwargs=None):
    import concourse.bacc as bacc
    import concourse.mybir as mybir
    import concourse.tile as tile
    from concourse.bass_utils import run_bass_kernel_spmd

    in_maps, meta = prep_host(**inputs)

    nc = bacc.Bacc("TRN2", target_bir_lowering=False, debug=False,
                   num_devices=NCORES, num_swdge_queues=4)
    f32 = mybir.dt.float32
    f32r = mybir.dt.float32r
    i16 = mybir.dt.int16
    tot = _layout(meta["groups"], meta["gpb"], meta["cap"])[2]

    ins = {}
    shapes = {
        "x0_out": ([meta["nodes_pad"], D], f32r),
        "x0_in": ([meta["nodes_pad"], D], f32r),
        "hfm": ([D, meta["shard"]], f32),
        "theta": ([D, D], f32),
        "ident": ([128, 128], f32),
        "idx0": ([128, tot // 16], i16),
        "denc0": ([128, tot // 128], f32),
        "val0": ([128, tot // 128], f32),
        "idx1": ([128, tot // 16], i16),
        "denc1": ([128, tot // 128], f32),
        "val1": ([128, tot // 128], f32),
    }
    for k, (shape, dt) in shapes.items():
        ins[k] = nc.dram_tensor(k, shape, dt, kind="ExternalInput").ap()
    y = nc.dram_tensor("y", [D, meta["shard"]], f32, kind="ExternalOutput")

    with tile.TileContext(nc) as tc:
        build_program(tc, ins, {"y": y.ap()}, meta)
    nc.compile()

    kw = {}
    if trace:
        kw = dict(trace=True, trace_kwargs=trace_kwargs or {})
    res = run_bass_kernel_spmd(nc, in_maps, core_ids=list(range(NCORES)),
                               **kw)
    shards = [r["y"].T for r in res.results]  # each [shard, 64]
    out = np.concatenate(shards, axis=0)[:N_NODES]
    return np.ascontiguousarray(out.astype(np.float32)), res



# revision 3
# speedup vs baseline: 1.5662x; 1.5662x over previous
"""CascadeGDCN (3-hop graph diffusion convolution) on 8 Trainium2 NeuronCores.

v2 design (vs the earlier baseline):
  - Destination nodes sharded across 8 cores (12544 rows each); edges
    partitioned by destination core; full X replicated per-core in DRAM and
    rebuilt by an AllGather after each hop (skipped after the last hop).
  - Variable-slot edge layout: per (128-dest group, 25088-source chunk) cell,
    slots = ceil(count/128) (shared across cores via per-cell max) instead of
    a uniform cap -> ~20% fewer gather descriptors.
  - Gather rate is the kernel bottleneck (Q7 SWDGE descriptor generation +
    4-queue drain, ~2.2 ns/row): 8-deep tile pools keep ~8 gather calls in
    flight across the 4 SWDGE queues.
  - bf16 compute path: S (one-hot x nothing) built by one DVE is_equal per
    call, edge values folded into the messages by one DVE multiply+cast;
    matmuls run bf16 (FWL weight loads + 1-pass streaming) instead of fp32r.
  - Segment reduction per group: PSUM [128 dests, 64 feat] accumulates
    lhsT=S (stationary) @ rhs=messages over the group's slots.
  - st accumulates in fp32 SBUF; new-X rows copied psum->SBUF on ScalarE and
    DMA'd per group straight to the bounce buffer (no big xnew SBUF tile).
"""

import numpy as np

D = 64
NCORES = 8
NUM_HOPS = 3
N_NODES = 100000
SHARD = 12544            # dest rows per core (98 groups of 128)
NODES_PAD = SHARD * NCORES   # 100352
NCHUNKS = 4
CHUNK = NODES_PAD // NCHUNKS  # 25088 (< 32768 so chunk-local idx fits int16)
GROUPS = SHARD // 128    # 98
GPB = 4                  # dest groups per block (per gather call)
BUFS = 8                 # stream-pool depth (gather pipelining)


def _softmax(x):
    e = np.exp(x - x.max())
    return e / e.sum()


def _blocks():
    out = []
    g = 0
    while g < GROUPS:
        out.append((g, min(GPB, GROUPS - g)))
        g += GPB
    return out


def _layout_from_slots(slots):
    """slots: [GROUPS, NCHUNKS] -> stream layout dicts.

    Stream order: for block b: for chunk c: for g in block: slots(g,c).
    Returns (blocks, O, ns, off_loc, ns_tot) with O/ns per (b,c) in slots,
    off_loc per (g,c) local slot offset inside its (b,c) call.
    """
    blocks = _blocks()
    O = {}
    ns = {}
    off_loc = {}
    pos = 0
    for bi, (g0, gc) in enumerate(blocks):
        for c in range(NCHUNKS):
            O[(bi, c)] = pos
            loc = 0
            for gl in range(gc):
                g = g0 + gl
                off_loc[(g, c)] = loc
                loc += int(slots[g, c])
            ns[(bi, c)] = loc
            pos += loc
    return blocks, O, ns, off_loc, pos


def _direction_counts(dest, src):
    """Per-core per-cell edge counts -> shared slots table (max over cores)."""
    counts = np.zeros((NCORES, GROUPS, NCHUNKS), dtype=np.int64)
    core = dest // SHARD
    for m in range(NCORES):
        sel = core == m
        d_loc = dest[sel] - m * SHARD
        g = d_loc >> 7
        c = src[sel] // CHUNK
        np.add.at(counts, (m, g, c), 1)
    slots = (np.max(counts, axis=0) + 127) // 128
    return slots


def _prep_direction(dest, src, val, slots, layout):
    """Build per-core gather/S tables for one SpMM direction."""
    blocks, O, ns, off_loc, ns_tot = layout
    cell_base = np.zeros(GROUPS * NCHUNKS, dtype=np.int64)
    for g in range(GROUPS):
        bi = g // GPB
        for c in range(NCHUNKS):
            cell_base[g * NCHUNKS + c] = O[(bi, c)] + off_loc[(g, c)]
    tot = ns_tot * 128

    core = dest // SHARD
    out = []
    for m in range(NCORES):
        sel = core == m
        d_loc = (dest[sel] - m * SHARD).astype(np.int64)
        s = src[sel].astype(np.int64)
        v = val[sel].astype(np.float32)
        g = d_loc >> 7
        c = s // CHUNK
        cell = g * NCHUNKS + c
        order = np.argsort(cell, kind="stable")
        cell_s = cell[order]
        counts = np.bincount(cell_s, minlength=GROUPS * NCHUNKS)
        starts = np.zeros(GROUPS * NCHUNKS, dtype=np.int64)
        starts[1:] = np.cumsum(counts)[:-1]
        rank = np.arange(cell_s.size) - starts[cell_s]
        pos = cell_base[cell_s] * 128 + rank

        idx_st = np.zeros(tot, dtype=np.int16)
        denc_st = np.full(tot, -1.0, dtype=np.float32)
        val_st = np.zeros(tot, dtype=np.float32)
        idx_st[pos] = (s[order] - c[order] * CHUNK).astype(np.int16)
        denc_st[pos] = (d_loc[order] & 127).astype(np.float32)
        val_st[pos] = v[order]

        # idx stream wrapped in 16 partitions, replicated into 8 Q7 stripes
        idx_tbl = np.tile(np.ascontiguousarray(idx_st.reshape(-1, 16).T),
                          (8, 1))
        denc_tbl = np.ascontiguousarray(denc_st.reshape(-1, 128).T)
        val_tbl = np.ascontiguousarray(val_st.reshape(-1, 128).T)
        out.append({"idx": idx_tbl, "denc": denc_tbl, "val": val_tbl})
    return out


def prep_host(H_l, edge_row, edge_col, edge_val, out_degree, in_degree,
              hop_attention, theta_out, theta_in, Theta):
    from ml_dtypes import bfloat16

    H = np.asarray(H_l, dtype=np.float32)
    er = np.asarray(edge_row, dtype=np.int64)
    ec = np.asarray(edge_col, dtype=np.int64)
    ev = np.asarray(edge_val, dtype=np.float32)
    od = np.asarray(out_degree, dtype=np.float32)
    idg = np.asarray(in_degree, dtype=np.float32)

    alpha = _softmax(np.asarray(hop_attention, dtype=np.float64))
    th_o = np.asarray(theta_out, dtype=np.float64)
    th_i = np.asarray(theta_in, dtype=np.float64)
    coef = [(float(alpha[k] * th_o[k]), float(alpha[k] * th_i[k]))
            for k in range(len(alpha))]

    # dir 0 ("out" chain): dest=row, src=col; dir 1: transposed
    slots0 = _direction_counts(er, ec)
    slots1 = _direction_counts(ec, er)
    lay0 = _layout_from_slots(slots0)
    lay1 = _layout_from_slots(slots1)
    t0 = _prep_direction(er, ec, ev, slots0, lay0)
    t1 = _prep_direction(ec, er, ev, slots1, lay1)

    x0o = np.zeros((NODES_PAD, D), dtype=np.float32)
    x0i = np.zeros((NODES_PAD, D), dtype=np.float32)
    x0o[:N_NODES] = np.maximum(od, 1e-8)[:, None] * H
    x0i[:N_NODES] = np.maximum(idg, 1e-8)[:, None] * H

    hpad = np.zeros((NODES_PAD, D), dtype=np.float32)
    hpad[:N_NODES] = H
    ident = np.eye(128, dtype=np.float32)
    theta = np.ascontiguousarray(np.asarray(Theta, dtype=np.float32)).astype(
        bfloat16)

    nsmax = max(
        max(lay0[2].values()),
        max(lay1[2].values()),
    )
    iota = np.tile(np.arange(128, dtype=np.float32), nsmax)[None, :].repeat(
        128, axis=0).astype(bfloat16)

    in_maps = []
    for m in range(NCORES):
        in_maps.append({
            "x0_out": x0o,
            "x0_in": x0i,
            "hfm": np.ascontiguousarray(hpad[m * SHARD:(m + 1) * SHARD].T),
            "theta": theta,
            "ident": ident,
            "iota": iota,
            "idx0": t0[m]["idx"],
            "denc0": t0[m]["denc"].astype(bfloat16),
            "val0": t0[m]["val"],
            "idx1": t1[m]["idx"],
            "denc1": t1[m]["denc"].astype(bfloat16),
            "val1": t1[m]["val"],
        })
    meta = {"coef": coef, "lay": [lay0, lay1], "nsmax": int(nsmax),
            "slots": [slots0, slots1]}
    return in_maps, meta


def build_program(tc, ins, outs, meta):
    """Emit the full SPMD program into TileContext tc."""
    import concourse.mybir as mybir

    nc = tc.nc
    f32 = mybir.dt.float32
    f32r = mybir.dt.float32r
    bf16 = mybir.dt.bfloat16
    i16 = mybir.dt.int16
    EQ, MUL, ADD = (mybir.AluOpType.is_equal, mybir.AluOpType.mult,
                    mybir.AluOpType.add)

    coef = meta["coef"]
    nsmax = meta["nsmax"]
    lays = meta["lay"]
    rg = [list(range(NCORES))]

    bounce = [nc.dram_tensor(f"bounce{d}", [SHARD, D], f32r,
                             kind="Internal") for d in range(2)]
    xbuf = [[nc.dram_tensor(f"xbuf{d}_{p}", [NODES_PAD, D], f32r,
                            kind="Internal", addr_space="Shared")
             for p in range(2)] for d in range(2)]

    tabs = [
        (ins["idx0"], ins["denc0"], ins["val0"]),
        (ins["idx1"], ins["denc1"], ins["val1"]),
    ]
    x0 = [ins["x0_out"], ins["x0_in"]]

    with (
        tc.tile_pool(name="const", bufs=1) as cpool,
        tc.tile_pool(name="work", bufs=1) as wpool,
        tc.tile_pool(name="stream", bufs=BUFS) as spool,
        tc.tile_pool(name="xc", bufs=4) as xpool,
        tc.tile_pool(name="fin", bufs=2) as fpool,
        tc.tile_pool(name="ps", bufs=4, space="PSUM") as pspool,
        tc.tile_pool(name="psf", bufs=2, space="PSUM") as psfpool,
    ):
        iota_s = cpool.tile([128, nsmax, 128], bf16, tag="iota")
        nc.sync.dma_start(iota_s[:], ins["iota"][:].rearrange(
            "p (s c) -> p s c", c=128))
        ident_s = cpool.tile([128, 128], f32, tag="ident")
        nc.sync.dma_start(ident_s[:], ins["ident"][:])
        theta_s = cpool.tile([64, D], bf16, tag="theta")
        nc.sync.dma_start(theta_s[:], ins["theta"][:])

        # resident per-direction denc (bf16) and val (f32) tables
        denc_res = []
        val_res = []
        for d in range(2):
            ns_tot = lays[d][4]
            dt_ = wpool.tile([128, ns_tot], bf16, tag=f"denc{d}")
            nc.sync.dma_start(dt_[:], tabs[d][1][:])
            vt_ = wpool.tile([128, ns_tot], f32, tag=f"val{d}")
            nc.sync.dma_start(vt_[:], tabs[d][2][:])
            denc_res.append(dt_)
            val_res.append(vt_)

        st = wpool.tile([128, GROUPS, D], f32, tag="st")
        nc.vector.memset(st[:], 0.0)

        for hop in range(NUM_HOPS):
            for dirn in range(2):
                blocks, O, ns_map, off_loc, ns_tot = lays[dirn]
                slots = meta["slots"][dirn]
                idx_d = tabs[dirn][0]
                xsrc = (x0[dirn] if hop == 0
                        else xbuf[dirn][(hop - 1) % 2].ap())
                xv = xsrc.rearrange("(c n) d -> c n d", n=CHUNK)

                for bi, (g0, gc) in enumerate(blocks):
                    tiles_c = {}
                    for c in range(NCHUNKS):
                        ns = ns_map[(bi, c)]
                        if ns == 0:
                            continue
                        o = O[(bi, c)]
                        idx_t = spool.tile([128, nsmax * 8], i16, tag="idx")
                        nc.sync.dma_start(
                            idx_t[:, :ns * 8],
                            idx_d[:, o * 8:(o + ns) * 8])
                        msgs = spool.tile([128, nsmax, D], f32r, tag="msgs")
                        nc.gpsimd.dma_gather(
                            out_ap=msgs[:, :ns, :],
                            in_ap=xv[c].bitcast(f32r),
                            idxs_ap=idx_t[:, :ns * 8],
                            num_idxs=ns * 128,
                            num_idxs_reg=ns * 128,
                            elem_size=D,
                            single_packet=False,
                            queue_num=c,
                        )
                        S = spool.tile([128, nsmax, 128], bf16, tag="S")
                        nc.vector.tensor_tensor(
                            out=S[:, :ns, :],
                            in0=iota_s[:, :ns, :],
                            in1=denc_res[dirn][:, o:o + ns].broadcast_to(
                                [128, ns, 128]),
                            op=EQ)
                        mbf = spool.tile([128, nsmax, D], bf16, tag="mbf")
                        nc.vector.tensor_tensor(
                            out=mbf[:, :ns, :],
                            in0=msgs[:, :ns, :].bitcast(f32),
                            in1=val_res[dirn][:, o:o + ns].broadcast_to(
                                [128, ns, D]),
                            op=MUL)
                        tiles_c[c] = (S, mbf)

                    for gl in range(gc):
                        g = g0 + gl
                        total = int(slots[g].sum())
                        if total == 0:
                            continue
                        ps = pspool.tile([128, D], f32, tag="ps")
                        j = 0
                        for c in range(NCHUNKS):
                            sc = int(slots[g, c])
                            if sc == 0:
                                continue
                            S, mbf = tiles_c[c]
                            base = off_loc[(g, c)]
                            for s in range(sc):
                                sl = base + s
                                nc.tensor.matmul(
                                    ps[:],
                                    lhsT=S[:, sl, :],
                                    rhs=mbf[:, sl, :],
                                    start=(j == 0),
                                    stop=(j == total - 1),
                                )
                                j += 1
                        nc.vector.scalar_tensor_tensor(
                            out=st[:, g, :], in0=ps[:],
                            scalar=coef[hop][dirn], in1=st[:, g, :],
                            op0=MUL, op1=ADD)
                        if hop < NUM_HOPS - 1:
                            xc = xpool.tile([128, D], f32r, tag="xc")
                            nc.scalar.copy(out=xc[:], in_=ps[:])
                            bounce_v = bounce[dirn].ap().rearrange(
                                "(g p) f -> p g f", p=128)
                            nc.sync.dma_start(bounce_v[:, g, :], xc[:])

                if hop < NUM_HOPS - 1:
                    nc.gpsimd.collective_compute(
                        "AllGather", mybir.AluOpType.bypass,
                        replica_groups=rg,
                        ins=[bounce[dirn].ap().opt()],
                        outs=[xbuf[dirn][hop % 2].ap().opt()],
                    )

        # final: y_fm = sigmoid(Theta^T @ st_fm) + H_fm, feature-major
        fchunks = [(i * 4, min(4, GROUPS - i * 4))
                   for i in range((GROUPS + 3) // 4)]
        for gs, gcnt in fchunks:
            width = gcnt * 128
            stfm = fpool.tile([64, width], bf16, tag="stfm")
            for j in range(gcnt):
                pt = psfpool.tile([64, 128], f32, tag="pt")
                nc.tensor.transpose(pt[:], st[:, gs + j, :], ident_s[:])
                nc.scalar.copy(out=stfm[:, j * 128:(j + 1) * 128], in_=pt[:])
            zp = psfpool.tile([64, width], f32, tag="zp")
            nc.tensor.matmul(zp[:], lhsT=theta_s[:], rhs=stfm[:],
                             start=True, stop=True)
            sg = fpool.tile([64, width], f32, tag="sg")
            nc.scalar.activation(sg[:], zp[:],
                                 mybir.ActivationFunctionType.Sigmoid)
            hf = fpool.tile([64, width], f32, tag="hf")
            nc.sync.dma_start(
                hf[:], ins["hfm"][:, gs * 128:gs * 128 + width])
            yt = fpool.tile([64, width], f32, tag="yt")
            nc.vector.tensor_tensor(out=yt[:], in0=sg[:], in1=hf[:], op=ADD)
            nc.sync.dma_start(
                outs["y"][:, gs * 128:gs * 128 + width], yt[:])


def kernel(**inputs) -> np.ndarray:
    return _run(inputs, trace=False)[0]


def kernel_traced(inputs, trace_kwargs=None):
    """Returns (output, BassKernelResults) with NTFF trace if available."""
    return _run(inputs, trace=True, trace_kwargs=trace_kwargs or {})


def _run(inputs, trace=False, trace_kwargs=None):
    import concourse.bacc as bacc
    import concourse.mybir as mybir
    import concourse.tile as tile
    from concourse.bass_utils import run_bass_kernel_spmd

    in_maps, meta = prep_host(**inputs)

    nc = bacc.Bacc("TRN2", target_bir_lowering=False, debug=False,
                   num_devices=NCORES, num_swdge_queues=4)
    f32 = mybir.dt.float32
    f32r = mybir.dt.float32r
    bf16 = mybir.dt.bfloat16
    i16 = mybir.dt.int16
    nsmax = meta["nsmax"]
    tot0 = meta["lay"][0][4]
    tot1 = meta["lay"][1][4]

    ins = {}
    shapes = {
        "x0_out": ([NODES_PAD, D], f32r),
        "x0_in": ([NODES_PAD, D], f32r),
        "hfm": ([D, SHARD], f32),
        "theta": ([D, D], bf16),
        "ident": ([128, 128], f32),
        "iota": ([128, nsmax * 128], bf16),
        "idx0": ([128, tot0 * 8], i16),
        "denc0": ([128, tot0], bf16),
        "val0": ([128, tot0], f32),
        "idx1": ([128, tot1 * 8], i16),
        "denc1": ([128, tot1], bf16),
        "val1": ([128, tot1], f32),
    }
    for k, (shape, dt) in shapes.items():
        ins[k] = nc.dram_tensor(k, shape, dt, kind="ExternalInput").ap()
    y = nc.dram_tensor("y", [D, SHARD], f32, kind="ExternalOutput")

    with tile.TileContext(nc) as tc:
        build_program(tc, ins, {"y": y.ap()}, meta)
    nc.compile()

    kw = {}
    if trace:
        kw = dict(trace=True, trace_kwargs=trace_kwargs or {})
    res = run_bass_kernel_spmd(nc, in_maps, core_ids=list(range(NCORES)),
                               **kw)
    shards = [r["y"].T for r in res.results]  # each [SHARD, 64]
    out = np.concatenate(shards, axis=0)[:N_NODES]
    return np.ascontiguousarray(out.astype(np.float32)), res


# revision 8
# speedup vs baseline: 1.7730x; 1.1320x over previous
"""CascadeGDCN (3-hop graph diffusion convolution) on 8 Trainium2 NeuronCores.

v2 design (vs the earlier baseline):
  - Destination nodes sharded across 8 cores (12544 rows each); edges
    partitioned by destination core; full X replicated per-core in DRAM and
    rebuilt by an AllGather after each hop (skipped after the last hop).
  - Variable-slot edge layout: per (128-dest group, 25088-source chunk) cell,
    slots = ceil(count/128) (shared across cores via per-cell max) instead of
    a uniform cap -> ~20% fewer gather descriptors.
  - Gather rate is the kernel bottleneck (Q7 SWDGE descriptor generation +
    4-queue drain, ~2.2 ns/row): 8-deep tile pools keep ~8 gather calls in
    flight across the 4 SWDGE queues.
  - bf16 compute path: S (one-hot x nothing) built by one DVE is_equal per
    call, edge values folded into the messages by one DVE multiply+cast;
    matmuls run bf16 (FWL weight loads + 1-pass streaming) instead of fp32r.
  - Segment reduction per group: PSUM [128 dests, 64 feat] accumulates
    lhsT=S (stationary) @ rhs=messages over the group's slots.
  - st accumulates in fp32 SBUF; new-X rows copied psum->SBUF on ScalarE and
    DMA'd per group straight to the bounce buffer (no big xnew SBUF tile).
"""

import numpy as np

D = 64
NCORES = 8
NUM_HOPS = 3
N_NODES = 100000
SHARD = 12544            # dest rows per core (98 groups of 128)
NODES_PAD = SHARD * NCORES   # 100352
NCHUNKS = 4
CHUNK = NODES_PAD // NCHUNKS  # 25088 (< 32768 so chunk-local idx fits int16)
GROUPS = SHARD // 128    # 98
GPB = 4                  # dest groups per block (per gather call)
BUFS = 8                 # stream-pool depth (gather pipelining)


def _softmax(x):
    e = np.exp(x - x.max())
    return e / e.sum()


def _blocks():
    out = []
    g = 0
    while g < GROUPS:
        out.append((g, min(GPB, GROUPS - g)))
        g += GPB
    return out


def _layout_from_slots(slots):
    """slots: [GROUPS, NCHUNKS] -> stream layout dicts.

    Stream order: for block b: for chunk c: for g in block: slots(g,c).
    Returns (blocks, O, ns, off_loc, ns_tot) with O/ns per (b,c) in slots,
    off_loc per (g,c) local slot offset inside its (b,c) call.
    """
    blocks = _blocks()
    O = {}
    ns = {}
    off_loc = {}
    pos = 0
    for bi, (g0, gc) in enumerate(blocks):
        for c in range(NCHUNKS):
            O[(bi, c)] = pos
            loc = 0
            for gl in range(gc):
                g = g0 + gl
                off_loc[(g, c)] = loc
                loc += int(slots[g, c])
            ns[(bi, c)] = loc
            pos += loc
    return blocks, O, ns, off_loc, pos


def _direction_counts(dest, src):
    """Per-core per-cell edge counts -> shared slots table (max over cores)."""
    counts = np.zeros((NCORES, GROUPS, NCHUNKS), dtype=np.int64)
    core = dest // SHARD
    for m in range(NCORES):
        sel = core == m
        d_loc = dest[sel] - m * SHARD
        g = d_loc >> 7
        c = src[sel] // CHUNK
        np.add.at(counts, (m, g, c), 1)
    slots = (np.max(counts, axis=0) + 127) // 128
    return slots


def _prep_direction(dest, src, val, slots, layout):
    """Build per-core gather/S tables for one SpMM direction."""
    blocks, O, ns, off_loc, ns_tot = layout
    cell_base = np.zeros(GROUPS * NCHUNKS, dtype=np.int64)
    for g in range(GROUPS):
        bi = g // GPB
        for c in range(NCHUNKS):
            cell_base[g * NCHUNKS + c] = O[(bi, c)] + off_loc[(g, c)]
    tot = ns_tot * 128

    core = dest // SHARD
    out = []
    for m in range(NCORES):
        sel = core == m
        d_loc = (dest[sel] - m * SHARD).astype(np.int64)
        s = src[sel].astype(np.int64)
        v = val[sel].astype(np.float32)
        g = d_loc >> 7
        c = s // CHUNK
        cell = g * NCHUNKS + c
        order = np.argsort(cell, kind="stable")
        cell_s = cell[order]
        counts = np.bincount(cell_s, minlength=GROUPS * NCHUNKS)
        starts = np.zeros(GROUPS * NCHUNKS, dtype=np.int64)
        starts[1:] = np.cumsum(counts)[:-1]
        rank = np.arange(cell_s.size) - starts[cell_s]
        pos = cell_base[cell_s] * 128 + rank

        idx_st = np.zeros(tot, dtype=np.int16)
        denc_st = np.full(tot, -1.0, dtype=np.float32)
        val_st = np.zeros(tot, dtype=np.float32)
        idx_st[pos] = (s[order] - c[order] * CHUNK).astype(np.int16)
        denc_st[pos] = (d_loc[order] & 127).astype(np.float32)
        val_st[pos] = v[order]

        # idx stream wrapped in 16 partitions, replicated into 8 Q7 stripes
        idx_tbl = np.tile(np.ascontiguousarray(idx_st.reshape(-1, 16).T),
                          (8, 1))
        denc_tbl = np.ascontiguousarray(denc_st.reshape(-1, 128).T)
        val_tbl = np.ascontiguousarray(val_st.reshape(-1, 128).T)
        out.append({"idx": idx_tbl, "denc": denc_tbl, "val": val_tbl})
    return out


def prep_host(H_l, edge_row, edge_col, edge_val, out_degree, in_degree,
              hop_attention, theta_out, theta_in, Theta):
    from ml_dtypes import bfloat16

    H = np.asarray(H_l, dtype=np.float32)
    er = np.asarray(edge_row, dtype=np.int64)
    ec = np.asarray(edge_col, dtype=np.int64)
    ev = np.asarray(edge_val, dtype=np.float32)
    od = np.asarray(out_degree, dtype=np.float32)
    idg = np.asarray(in_degree, dtype=np.float32)

    alpha = _softmax(np.asarray(hop_attention, dtype=np.float64))
    th_o = np.asarray(theta_out, dtype=np.float64)
    th_i = np.asarray(theta_in, dtype=np.float64)
    coef = [(float(alpha[k] * th_o[k]), float(alpha[k] * th_i[k]))
            for k in range(len(alpha))]

    # dir 0 ("out" chain): dest=row, src=col; dir 1: transposed
    slots0 = _direction_counts(er, ec)
    slots1 = _direction_counts(ec, er)
    lay0 = _layout_from_slots(slots0)
    lay1 = _layout_from_slots(slots1)
    t0 = _prep_direction(er, ec, ev, slots0, lay0)
    t1 = _prep_direction(ec, er, ev, slots1, lay1)

    x0o = np.zeros((NODES_PAD, D), dtype=np.float32)
    x0i = np.zeros((NODES_PAD, D), dtype=np.float32)
    x0o[:N_NODES] = np.maximum(od, 1e-8)[:, None] * H
    x0i[:N_NODES] = np.maximum(idg, 1e-8)[:, None] * H

    hpad = np.zeros((NODES_PAD, D), dtype=np.float32)
    hpad[:N_NODES] = H
    ident = np.eye(128, dtype=np.float32)
    theta = np.ascontiguousarray(np.asarray(Theta, dtype=np.float32)).astype(
        bfloat16)

    nsmax = max(
        max(lay0[2].values()),
        max(lay1[2].values()),
    )
    iota = np.tile(np.arange(128, dtype=np.float32), nsmax)[None, :].repeat(
        128, axis=0).astype(bfloat16)

    in_maps = []
    for m in range(NCORES):
        in_maps.append({
            "x0_out": x0o,
            "x0_in": x0i,
            "hfm": np.ascontiguousarray(hpad[m * SHARD:(m + 1) * SHARD].T),
            "theta": theta,
            "ident": ident,
            "iota": iota,
            "idx0": t0[m]["idx"],
            "denc0": t0[m]["denc"].astype(bfloat16),
            "val0": t0[m]["val"],
            "idx1": t1[m]["idx"],
            "denc1": t1[m]["denc"].astype(bfloat16),
            "val1": t1[m]["val"],
        })
    meta = {"coef": coef, "lay": [lay0, lay1], "nsmax": int(nsmax),
            "slots": [slots0, slots1]}
    return in_maps, meta


def build_program(tc, ins, outs, meta):
    """Emit the full SPMD program into TileContext tc."""
    import concourse.mybir as mybir

    nc = tc.nc
    f32 = mybir.dt.float32
    f32r = mybir.dt.float32r
    bf16 = mybir.dt.bfloat16
    i16 = mybir.dt.int16
    EQ, MUL, ADD = (mybir.AluOpType.is_equal, mybir.AluOpType.mult,
                    mybir.AluOpType.add)

    coef = meta["coef"]
    nsmax = meta["nsmax"]
    lays = meta["lay"]
    rg = [list(range(NCORES))]

    bounce = [nc.dram_tensor(f"bounce{d}", [SHARD, D], f32r,
                             kind="Internal") for d in range(2)]
    xbuf = [[nc.dram_tensor(f"xbuf{d}_{p}", [NODES_PAD, D], f32r,
                            kind="Internal", addr_space="Shared")
             for p in range(2)] for d in range(2)]

    tabs = [
        (ins["idx0"], ins["denc0"], ins["val0"]),
        (ins["idx1"], ins["denc1"], ins["val1"]),
    ]
    x0 = [ins["x0_out"], ins["x0_in"]]

    with (
        tc.tile_pool(name="const", bufs=1) as cpool,
        tc.tile_pool(name="work", bufs=1) as wpool,
        tc.tile_pool(name="stream", bufs=BUFS) as spool,
        tc.tile_pool(name="smat", bufs=5) as spool2,
        tc.tile_pool(name="xc", bufs=4) as xpool,
        tc.tile_pool(name="fin", bufs=1) as fpool,
        tc.tile_pool(name="ps", bufs=4, space="PSUM") as pspool,
        tc.tile_pool(name="psf", bufs=2, space="PSUM") as psfpool,
    ):
        iota_s = cpool.tile([128, nsmax, 128], bf16, tag="iota")
        nc.sync.dma_start(iota_s[:], ins["iota"][:].rearrange(
            "p (s c) -> p s c", c=128))
        ident_s = cpool.tile([128, 128], f32, tag="ident")
        nc.sync.dma_start(ident_s[:], ins["ident"][:])
        theta_s = cpool.tile([64, D], bf16, tag="theta")
        nc.sync.dma_start(theta_s[:], ins["theta"][:])

        # resident per-direction idx / denc (bf16) / val (f32) tables
        denc_res = []
        val_res = []
        idx_res = []
        for d in range(2):
            ns_tot = lays[d][4]
            it_ = wpool.tile([128, ns_tot * 8], i16, tag=f"idx{d}")
            nc.sync.dma_start(it_[:], tabs[d][0][:])
            dt_ = wpool.tile([128, ns_tot], bf16, tag=f"denc{d}")
            nc.sync.dma_start(dt_[:], tabs[d][1][:])
            vt_ = wpool.tile([128, ns_tot], f32, tag=f"val{d}")
            nc.sync.dma_start(vt_[:], tabs[d][2][:])
            idx_res.append(it_)
            denc_res.append(dt_)
            val_res.append(vt_)

        st = wpool.tile([128, GROUPS, D], f32, tag="st")
        nc.vector.memset(st[:], 0.0)

        for hop in range(NUM_HOPS):
            for dirn in range(2):
                blocks, O, ns_map, off_loc, ns_tot = lays[dirn]
                slots = meta["slots"][dirn]
                xsrc = (x0[dirn] if hop == 0
                        else xbuf[dirn][(hop - 1) % 2].ap())
                xv = xsrc.rearrange("(c n) d -> c n d", n=CHUNK)

                for bi, (g0, gc) in enumerate(blocks):
                    tiles_c = {}
                    for c in range(NCHUNKS):
                        ns = ns_map[(bi, c)]
                        if ns == 0:
                            continue
                        o = O[(bi, c)]
                        msgs = spool.tile([128, nsmax, D], f32r, tag="msgs")
                        nc.gpsimd.dma_gather(
                            out_ap=msgs[:, :ns, :],
                            in_ap=xv[c].bitcast(f32r),
                            idxs_ap=idx_res[dirn][:, o * 8:(o + ns) * 8],
                            num_idxs=ns * 128,
                            num_idxs_reg=ns * 128,
                            elem_size=D,
                            single_packet=False,
                            queue_num=c,
                        )
                        S = spool2.tile([128, nsmax, 128], bf16, tag="S")
                        nc.vector.tensor_tensor(
                            out=S[:, :ns, :],
                            in0=iota_s[:, :ns, :],
                            in1=denc_res[dirn][:, o:o + ns].broadcast_to(
                                [128, ns, 128]),
                            op=EQ)
                        mbf = spool2.tile([128, nsmax, D], bf16, tag="mbf")
                        nc.vector.tensor_tensor(
                            out=mbf[:, :ns, :],
                            in0=msgs[:, :ns, :].bitcast(f32),
                            in1=val_res[dirn][:, o:o + ns].broadcast_to(
                                [128, ns, D]),
                            op=MUL)
                        tiles_c[c] = (S, mbf)

                    for gl in range(gc):
                        g = g0 + gl
                        total = int(slots[g].sum())
                        if total == 0:
                            continue
                        ps = pspool.tile([128, D], f32, tag="ps")
                        j = 0
                        for c in range(NCHUNKS):
                            sc = int(slots[g, c])
                            if sc == 0:
                                continue
                            S, mbf = tiles_c[c]
                            base = off_loc[(g, c)]
                            for s in range(sc):
                                sl = base + s
                                nc.tensor.matmul(
                                    ps[:],
                                    lhsT=S[:, sl, :],
                                    rhs=mbf[:, sl, :],
                                    start=(j == 0),
                                    stop=(j == total - 1),
                                )
                                j += 1
                        nc.vector.scalar_tensor_tensor(
                            out=st[:, g, :], in0=ps[:],
                            scalar=coef[hop][dirn], in1=st[:, g, :],
                            op0=MUL, op1=ADD)
                        if hop < NUM_HOPS - 1:
                            xc = xpool.tile([128, D], f32r, tag="xc")
                            nc.scalar.copy(out=xc[:], in_=ps[:])
                            bounce_v = bounce[dirn].ap().rearrange(
                                "(g p) f -> p g f", p=128)
                            nc.sync.dma_start(bounce_v[:, g, :], xc[:])

                if hop < NUM_HOPS - 1:
                    nc.gpsimd.collective_compute(
                        "AllGather", mybir.AluOpType.bypass,
                        replica_groups=rg,
                        ins=[bounce[dirn].ap().opt()],
                        outs=[xbuf[dirn][hop % 2].ap().opt()],
                    )

        # final: y_fm = sigmoid(Theta^T @ st_fm) + H_fm, feature-major
        fchunks = [(i * 4, min(4, GROUPS - i * 4))
                   for i in range((GROUPS + 3) // 4)]
        for gs, gcnt in fchunks:
            width = gcnt * 128
            stfm = fpool.tile([64, width], bf16, tag="stfm")
            for j in range(gcnt):
                pt = psfpool.tile([64, 128], f32, tag="pt")
                nc.tensor.transpose(pt[:], st[:, gs + j, :], ident_s[:])
                nc.scalar.copy(out=stfm[:, j * 128:(j + 1) * 128], in_=pt[:])
            zp = psfpool.tile([64, width], f32, tag="zp")
            nc.tensor.matmul(zp[:], lhsT=theta_s[:], rhs=stfm[:],
                             start=True, stop=True)
            sg = fpool.tile([64, width], f32, tag="sg")
            nc.scalar.activation(sg[:], zp[:],
                                 mybir.ActivationFunctionType.Sigmoid)
            hf = fpool.tile([64, width], f32, tag="hf")
            nc.sync.dma_start(
                hf[:], ins["hfm"][:, gs * 128:gs * 128 + width])
            yt = fpool.tile([64, width], f32, tag="yt")
            nc.vector.tensor_tensor(out=yt[:], in0=sg[:], in1=hf[:], op=ADD)
            nc.sync.dma_start(
                outs["y"][:, gs * 128:gs * 128 + width], yt[:])


def kernel(**inputs) -> np.ndarray:
    return _run(inputs, trace=False)[0]


def kernel_traced(inputs, trace_kwargs=None):
    """Returns (output, BassKernelResults) with NTFF trace if available."""
    return _run(inputs, trace=True, trace_kwargs=trace_kwargs or {})


def _run(inputs, trace=False, trace_kwargs=None):
    import concourse.bacc as bacc
    import concourse.mybir as mybir
    import concourse.tile as tile
    from concourse.bass_utils import run_bass_kernel_spmd

    in_maps, meta = prep_host(**inputs)

    nc = bacc.Bacc("TRN2", target_bir_lowering=False, debug=False,
                   num_devices=NCORES, num_swdge_queues=4)
    f32 = mybir.dt.float32
    f32r = mybir.dt.float32r
    bf16 = mybir.dt.bfloat16
    i16 = mybir.dt.int16
    nsmax = meta["nsmax"]
    tot0 = meta["lay"][0][4]
    tot1 = meta["lay"][1][4]

    ins = {}
    shapes = {
        "x0_out": ([NODES_PAD, D], f32r),
        "x0_in": ([NODES_PAD, D], f32r),
        "hfm": ([D, SHARD], f32),
        "theta": ([D, D], bf16),
        "ident": ([128, 128], f32),
        "iota": ([128, nsmax * 128], bf16),
        "idx0": ([128, tot0 * 8], i16),
        "denc0": ([128, tot0], bf16),
        "val0": ([128, tot0], f32),
        "idx1": ([128, tot1 * 8], i16),
        "denc1": ([128, tot1], bf16),
        "val1": ([128, tot1], f32),
    }
    for k, (shape, dt) in shapes.items():
        ins[k] = nc.dram_tensor(k, shape, dt, kind="ExternalInput").ap()
    y = nc.dram_tensor("y", [D, SHARD], f32, kind="ExternalOutput")

    with tile.TileContext(nc) as tc:
        build_program(tc, ins, {"y": y.ap()}, meta)
    nc.compile()

    kw = {}
    if trace:
        kw = dict(trace=True, trace_kwargs=trace_kwargs or {})
    res = run_bass_kernel_spmd(nc, in_maps, core_ids=list(range(NCORES)),
                               **kw)
    shards = [r["y"].T for r in res.results]  # each [SHARD, 64]
    out = np.concatenate(shards, axis=0)[:N_NODES]
    return np.ascontiguousarray(out.astype(np.float32)), res


# revision 9
# speedup vs baseline: 1.9311x; 1.0892x over previous
"""CascadeGDCN (3-hop graph diffusion convolution) on 8 Trainium2 NeuronCores.

v4 design:
  - Destination nodes sharded across 8 cores (12544 rows each); edges
    partitioned by destination core; full X replicated per-core in DRAM and
    rebuilt by an AllGather after each hop (skipped after the last hop).
  - Packed-call edge layout: per gather call (4-dest-group block x source
    chunk) the 4 groups' edge segments are packed back-to-back at LANE
    granularity (per-group length = max edge count over the 8 cores, so the
    SPMD program structure is shared); slots of 128 edges may straddle a
    group boundary.  Boundary slots get two one-hot S columns (one per
    group).  This removes most of the per-(group,chunk) ceil padding that a
    slot-aligned layout pays -> ~14% fewer gather descriptors.
  - The gather (Q7 SWDGE descriptor generation at ~2.3 ns/row across 4
    queues) is the kernel bottleneck; idx tables are SBUF-resident and 8
    message buffers keep the gather queues saturated.
  - bf16 compute: S one-hot built by one DVE is_equal per call (vs resident
    denc), edge values folded into messages by one DVE multiply+cast,
    matmuls bf16 (FWL weight load + 1-pass streaming), PSUM accumulates
    [128 dests, 64 feat] per group.
  - st accumulates fp32 in SBUF; new-X rows go psum -> SBUF (ScalarE) ->
    per-group DMA into the bounce buffer feeding the AllGather.
  - The final stage (transpose, Theta matmul, sigmoid, +H) is emitted
    per-block inside the last SpMM so it overlaps the tail of the gathers.
"""

import numpy as np

D = 64
NCORES = 8
NUM_HOPS = 3
N_NODES = 100000
SHARD = 12544            # dest rows per core (98 groups of 128)
NODES_PAD = SHARD * NCORES   # 100352
NCHUNKS = 4
CHUNK = NODES_PAD // NCHUNKS  # 25088 (< 32768 so chunk-local idx fits int16)
GROUPS = SHARD // 128    # 98
GPB = 4                  # dest groups per block (per gather call)
BUFS = 8                 # message-tile pool depth (gather pipelining)


def _softmax(x):
    e = np.exp(x - x.max())
    return e / e.sum()


def _blocks():
    out = []
    g = 0
    while g < GROUPS:
        out.append((g, min(GPB, GROUPS - g)))
        g += GPB
    return out


def _direction_layout(dest, src):
    """Shared (SPMD) packed-call layout for one direction.

    Returns dict with:
      maxc[g, c]      per-cell max edge count over cores
      seg[(g, c)]     lane offset of group g's segment inside call (b, c)
      nreal[(b, c)]   real slots per call;  Oreal[(b, c)] global real offset
      next_[(b, c)]   ext (S) slots per call; Oext[(b, c)] global ext offset
      ents[(b, c)]    list of (slot_local, g, lane_lo, lane_hi) ext entries
      sched[g]        list of (c, ext_local, real_local) matmuls for group g
      nreal_tot, next_tot
    """
    counts = np.zeros((NCORES, GROUPS, NCHUNKS), dtype=np.int64)
    core = dest // SHARD
    for m in range(NCORES):
        sel = core == m
        d_loc = dest[sel] - m * SHARD
        g = d_loc >> 7
        c = src[sel] // CHUNK
        np.add.at(counts, (m, g, c), 1)
    maxc = np.max(counts, axis=0)

    blocks = _blocks()
    seg = {}
    nreal = {}
    next_ = {}
    Oreal = {}
    Oext = {}
    ents = {}
    sched = {g: [] for g in range(GROUPS)}
    pr = 0
    pe = 0
    for bi, (g0, gc) in enumerate(blocks):
        for c in range(NCHUNKS):
            lane = 0
            lo_hi = []
            for gl in range(gc):
                g = g0 + gl
                seg[(g, c)] = lane
                lo_hi.append((g, lane, lane + int(maxc[g, c])))
                lane += int(maxc[g, c])
            ns = (lane + 127) // 128
            nreal[(bi, c)] = ns
            Oreal[(bi, c)] = pr
            pr += ns
            # ext entries: per slot, per overlapping group
            Oext[(bi, c)] = pe
            el = []
            for s in range(ns):
                s_lo, s_hi = s * 128, (s + 1) * 128
                for g, a, b in lo_hi:
                    lo = max(s_lo, a)
                    hi = min(s_hi, b)
                    if lo < hi:
                        el.append((s, g, lo - s_lo, hi - s_lo))
                        sched[g].append((c, len(el) - 1 + pe - Oext[(bi, c)],
                                         s))
            ents[(bi, c)] = el
            next_[(bi, c)] = len(el)
            pe += len(el)
    return {"maxc": maxc, "seg": seg, "nreal": nreal, "next": next_,
            "Oreal": Oreal, "Oext": Oext, "ents": ents, "sched": sched,
            "nreal_tot": pr, "next_tot": pe, "blocks": blocks}


def _prep_direction(dest, src, val, lay):
    """Per-core idx/val (real-slot stream) and denc (ext stream) tables."""
    maxc = lay["maxc"]
    seg = lay["seg"]
    nreal = lay["nreal"]
    Oreal = lay["Oreal"]
    Oext = lay["Oext"]
    ents = lay["ents"]
    blocks = lay["blocks"]
    tot_r = lay["nreal_tot"] * 128
    tot_e = lay["next_tot"]

    # per-cell global lane base = call real base*128 + segment offset
    cell_base = np.zeros(GROUPS * NCHUNKS, dtype=np.int64)
    for g in range(GROUPS):
        bi = g // GPB
        for c in range(NCHUNKS):
            cell_base[g * NCHUNKS + c] = Oreal[(bi, c)] * 128 + seg[(g, c)]

    core = dest // SHARD
    out = []
    for m in range(NCORES):
        sel = core == m
        d_loc = (dest[sel] - m * SHARD).astype(np.int64)
        s = src[sel].astype(np.int64)
        v = val[sel].astype(np.float32)
        g = d_loc >> 7
        c = s // CHUNK
        cell = g * NCHUNKS + c
        order = np.argsort(cell, kind="stable")
        cell_s = cell[order]
        counts = np.bincount(cell_s, minlength=GROUPS * NCHUNKS)
        starts = np.zeros(GROUPS * NCHUNKS, dtype=np.int64)
        starts[1:] = np.cumsum(counts)[:-1]
        rank = np.arange(cell_s.size) - starts[cell_s]
        pos = cell_base[cell_s] + rank

        idx_st = np.zeros(tot_r, dtype=np.int16)
        denc_lane = np.full(tot_r, -1.0, dtype=np.float32)
        val_st = np.zeros(tot_r, dtype=np.float32)
        idx_st[pos] = (s[order] - c[order] * CHUNK).astype(np.int16)
        denc_lane[pos] = (d_loc[order] & 127).astype(np.float32)
        val_st[pos] = v[order]

        # ext denc stream: per ext entry, group lanes only, -1 elsewhere
        denc_ext = np.full((tot_e, 128), -1.0, dtype=np.float32)
        for bi, (g0, gc) in enumerate(blocks):
            for c in range(NCHUNKS):
                ob = Oreal[(bi, c)] * 128
                oe = Oext[(bi, c)]
                for k, (sl, g, lo, hi) in enumerate(ents[(bi, c)]):
                    denc_ext[oe + k, lo:hi] = denc_lane[
                        ob + sl * 128 + lo: ob + sl * 128 + hi]

        idx_tbl = np.tile(np.ascontiguousarray(idx_st.reshape(-1, 16).T),
                          (8, 1))
        denc_tbl = np.ascontiguousarray(denc_ext.T)
        val_tbl = np.ascontiguousarray(val_st.reshape(-1, 128).T)
        out.append({"idx": idx_tbl, "denc": denc_tbl, "val": val_tbl})
    return out


def prep_host(H_l, edge_row, edge_col, edge_val, out_degree, in_degree,
              hop_attention, theta_out, theta_in, Theta):
    from ml_dtypes import bfloat16

    H = np.asarray(H_l, dtype=np.float32)
    er = np.asarray(edge_row, dtype=np.int64)
    ec = np.asarray(edge_col, dtype=np.int64)
    ev = np.asarray(edge_val, dtype=np.float32)
    od = np.asarray(out_degree, dtype=np.float32)
    idg = np.asarray(in_degree, dtype=np.float32)

    alpha = _softmax(np.asarray(hop_attention, dtype=np.float64))
    th_o = np.asarray(theta_out, dtype=np.float64)
    th_i = np.asarray(theta_in, dtype=np.float64)
    coef = [(float(alpha[k] * th_o[k]), float(alpha[k] * th_i[k]))
            for k in range(len(alpha))]

    lay0 = _direction_layout(er, ec)
    lay1 = _direction_layout(ec, er)
    t0 = _prep_direction(er, ec, ev, lay0)
    t1 = _prep_direction(ec, er, ev, lay1)

    x0o = np.zeros((NODES_PAD, D), dtype=np.float32)
    x0i = np.zeros((NODES_PAD, D), dtype=np.float32)
    x0o[:N_NODES] = np.maximum(od, 1e-8)[:, None] * H
    x0i[:N_NODES] = np.maximum(idg, 1e-8)[:, None] * H

    hpad = np.zeros((NODES_PAD, D), dtype=np.float32)
    hpad[:N_NODES] = H
    ident = np.eye(128, dtype=np.float32)
    theta = np.ascontiguousarray(np.asarray(Theta, dtype=np.float32)).astype(
        bfloat16)

    nsmax = 0
    nemax = 0
    for lay in (lay0, lay1):
        nsmax = max(nsmax, max(lay["nreal"].values()))
        nemax = max(nemax, max(lay["next"].values()))
    iota = np.tile(np.arange(128, dtype=np.float32), nemax)[None, :].repeat(
        128, axis=0).astype(bfloat16)

    in_maps = []
    for m in range(NCORES):
        in_maps.append({
            "x0_out": x0o,
            "x0_in": x0i,
            "hfm": np.ascontiguousarray(hpad[m * SHARD:(m + 1) * SHARD].T),
            "theta": theta,
            "ident": ident,
            "iota": iota,
            "idx0": t0[m]["idx"],
            "denc0": t0[m]["denc"].astype(bfloat16),
            "val0": t0[m]["val"],
            "idx1": t1[m]["idx"],
            "denc1": t1[m]["denc"].astype(bfloat16),
            "val1": t1[m]["val"],
        })
    meta = {"coef": coef, "lay": [lay0, lay1], "nsmax": int(nsmax),
            "nemax": int(nemax)}
    return in_maps, meta


def build_program(tc, ins, outs, meta):
    """Emit the full SPMD program into TileContext tc."""
    import concourse.mybir as mybir

    nc = tc.nc
    f32 = mybir.dt.float32
    f32r = mybir.dt.float32r
    bf16 = mybir.dt.bfloat16
    i16 = mybir.dt.int16
    EQ, MUL, ADD = (mybir.AluOpType.is_equal, mybir.AluOpType.mult,
                    mybir.AluOpType.add)

    coef = meta["coef"]
    nsmax = meta["nsmax"]
    nemax = meta["nemax"]
    lays = meta["lay"]
    rg = [list(range(NCORES))]

    bounce = [nc.dram_tensor(f"bounce{d}", [SHARD, D], f32r,
                             kind="Internal") for d in range(2)]
    xbuf = [[nc.dram_tensor(f"xbuf{d}_{p}", [NODES_PAD, D], f32r,
                            kind="Internal", addr_space="Shared")
             for p in range(2)] for d in range(2)]

    tabs = [
        (ins["idx0"], ins["denc0"], ins["val0"]),
        (ins["idx1"], ins["denc1"], ins["val1"]),
    ]
    x0 = [ins["x0_out"], ins["x0_in"]]

    with (
        tc.tile_pool(name="const", bufs=1) as cpool,
        tc.tile_pool(name="work", bufs=1) as wpool,
        tc.tile_pool(name="stream", bufs=BUFS) as spool,
        tc.tile_pool(name="smat", bufs=5) as spool2,
        tc.tile_pool(name="xc", bufs=4) as xpool,
        tc.tile_pool(name="fin", bufs=2) as fpool,
        tc.tile_pool(name="ps", bufs=4, space="PSUM") as pspool,
        tc.tile_pool(name="psf", bufs=2, space="PSUM") as psfpool,
    ):
        iota_s = cpool.tile([128, nemax, 128], bf16, tag="iota")
        nc.sync.dma_start(iota_s[:], ins["iota"][:].rearrange(
            "p (s c) -> p s c", c=128))
        ident_s = cpool.tile([128, 128], f32, tag="ident")
        nc.sync.dma_start(ident_s[:], ins["ident"][:])
        theta_s = cpool.tile([64, D], bf16, tag="theta")
        nc.sync.dma_start(theta_s[:], ins["theta"][:])

        # resident per-direction idx / denc (bf16, ext) / val (f32) tables
        denc_res = []
        val_res = []
        idx_res = []
        for d in range(2):
            it_ = wpool.tile([128, lays[d]["nreal_tot"] * 8], i16,
                             tag=f"idx{d}")
            nc.sync.dma_start(it_[:], tabs[d][0][:])
            dt_ = wpool.tile([128, lays[d]["next_tot"]], bf16, tag=f"denc{d}")
            nc.sync.dma_start(dt_[:], tabs[d][1][:])
            vt_ = wpool.tile([128, lays[d]["nreal_tot"]], f32, tag=f"val{d}")
            nc.sync.dma_start(vt_[:], tabs[d][2][:])
            idx_res.append(it_)
            denc_res.append(dt_)
            val_res.append(vt_)

        st = wpool.tile([128, GROUPS, D], f32, tag="st")
        nc.vector.memset(st[:], 0.0)

        def emit_final(gs, gcnt):
            width = gcnt * 128
            stfm = fpool.tile([64, GPB * 128], bf16, tag="stfm")
            for j in range(gcnt):
                pt = psfpool.tile([64, 128], f32, tag="pt")
                nc.tensor.transpose(pt[:], st[:, gs + j, :], ident_s[:])
                nc.scalar.copy(out=stfm[:, j * 128:(j + 1) * 128], in_=pt[:])
            zp = psfpool.tile([64, GPB * 128], f32, tag="zp")
            nc.tensor.matmul(zp[:, :width], lhsT=theta_s[:],
                             rhs=stfm[:, :width], start=True, stop=True)
            sg = fpool.tile([64, GPB * 128], f32, tag="sg")
            nc.scalar.activation(sg[:, :width], zp[:, :width],
                                 mybir.ActivationFunctionType.Sigmoid)
            hf = fpool.tile([64, GPB * 128], f32, tag="hf")
            nc.sync.dma_start(
                hf[:, :width], ins["hfm"][:, gs * 128:gs * 128 + width])
            yt = fpool.tile([64, GPB * 128], f32, tag="yt")
            nc.vector.tensor_tensor(out=yt[:, :width], in0=sg[:, :width],
                                    in1=hf[:, :width], op=ADD)
            nc.sync.dma_start(
                outs["y"][:, gs * 128:gs * 128 + width], yt[:, :width])

        for hop in range(NUM_HOPS):
            for dirn in range(2):
                lay = lays[dirn]
                blocks = lay["blocks"]
                sched = lay["sched"]
                xsrc = (x0[dirn] if hop == 0
                        else xbuf[dirn][(hop - 1) % 2].ap())
                xv = xsrc.rearrange("(c n) d -> c n d", n=CHUNK)
                last = hop == NUM_HOPS - 1 and dirn == 1

                for bi, (g0, gc) in enumerate(blocks):
                    tiles_c = {}
                    for c in range(NCHUNKS):
                        ns = lay["nreal"][(bi, c)]
                        ne = lay["next"][(bi, c)]
                        if ns == 0:
                            continue
                        o = lay["Oreal"][(bi, c)]
                        oe = lay["Oext"][(bi, c)]
                        msgs = spool.tile([128, nsmax, D], f32r, tag="msgs")
                        nc.gpsimd.dma_gather(
                            out_ap=msgs[:, :ns, :],
                            in_ap=xv[c].bitcast(f32r),
                            idxs_ap=idx_res[dirn][:, o * 8:(o + ns) * 8],
                            num_idxs=ns * 128,
                            num_idxs_reg=ns * 128,
                            elem_size=D,
                            single_packet=False,
                            queue_num=c,
                        )
                        S = spool2.tile([128, nemax, 128], bf16, tag="S")
                        nc.vector.tensor_tensor(
                            out=S[:, :ne, :],
                            in0=iota_s[:, :ne, :],
                            in1=denc_res[dirn][:, oe:oe + ne].broadcast_to(
                                [128, ne, 128]),
                            op=EQ)
                        mbf = spool2.tile([128, nsmax, D], bf16, tag="mbf")
                        nc.vector.tensor_tensor(
                            out=mbf[:, :ns, :],
                            in0=msgs[:, :ns, :].bitcast(f32),
                            in1=val_res[dirn][:, o:o + ns].broadcast_to(
                                [128, ns, D]),
                            op=MUL)
                        tiles_c[c] = (S, mbf)

                    for gl in range(gc):
                        g = g0 + gl
                        mms = sched[g]
                        if not mms:
                            continue
                        ps = pspool.tile([128, D], f32, tag="ps")
                        for j, (c, exl, rel) in enumerate(mms):
                            S, mbf = tiles_c[c]
                            nc.tensor.matmul(
                                ps[:],
                                lhsT=S[:, exl, :],
                                rhs=mbf[:, rel, :],
                                start=(j == 0),
                                stop=(j == len(mms) - 1),
                            )
                        nc.vector.scalar_tensor_tensor(
                            out=st[:, g, :], in0=ps[:],
                            scalar=coef[hop][dirn], in1=st[:, g, :],
                            op0=MUL, op1=ADD)
                        if hop < NUM_HOPS - 1:
                            xc = xpool.tile([128, D], f32r, tag="xc")
                            nc.scalar.copy(out=xc[:], in_=ps[:])
                            bounce_v = bounce[dirn].ap().rearrange(
                                "(g p) f -> p g f", p=128)
                            nc.sync.dma_start(bounce_v[:, g, :], xc[:])
                    if last:
                        emit_final(g0, gc)

                if hop < NUM_HOPS - 1:
                    nc.gpsimd.collective_compute(
                        "AllGather", mybir.AluOpType.bypass,
                        replica_groups=rg,
                        ins=[bounce[dirn].ap().opt()],
                        outs=[xbuf[dirn][hop % 2].ap().opt()],
                    )


def kernel(**inputs) -> np.ndarray:
    return _run(inputs, trace=False)[0]


def kernel_traced(inputs, trace_kwargs=None):
    """Returns (output, BassKernelResults) with NTFF trace if available."""
    return _run(inputs, trace=True, trace_kwargs=trace_kwargs or {})


def _run(inputs, trace=False, trace_kwargs=None):
    import concourse.bacc as bacc
    import concourse.mybir as mybir
    import concourse.tile as tile
    from concourse.bass_utils import run_bass_kernel_spmd

    in_maps, meta = prep_host(**inputs)

    nc = bacc.Bacc("TRN2", target_bir_lowering=False, debug=False,
                   num_devices=NCORES, num_swdge_queues=4)
    f32 = mybir.dt.float32
    f32r = mybir.dt.float32r
    bf16 = mybir.dt.bfloat16
    i16 = mybir.dt.int16
    nsmax = meta["nsmax"]
    nemax = meta["nemax"]
    r0 = meta["lay"][0]["nreal_tot"]
    e0 = meta["lay"][0]["next_tot"]
    r1 = meta["lay"][1]["nreal_tot"]
    e1 = meta["lay"][1]["next_tot"]

    ins = {}
    shapes = {
        "x0_out": ([NODES_PAD, D], f32r),
        "x0_in": ([NODES_PAD, D], f32r),
        "hfm": ([D, SHARD], f32),
        "theta": ([D, D], bf16),
        "ident": ([128, 128], f32),
        "iota": ([128, nemax * 128], bf16),
        "idx0": ([128, r0 * 8], i16),
        "denc0": ([128, e0], bf16),
        "val0": ([128, r0], f32),
        "idx1": ([128, r1 * 8], i16),
        "denc1": ([128, e1], bf16),
        "val1": ([128, r1], f32),
    }
    for k, (shape, dt) in shapes.items():
        ins[k] = nc.dram_tensor(k, shape, dt, kind="ExternalInput").ap()
    y = nc.dram_tensor("y", [D, SHARD], f32, kind="ExternalOutput")

    with tile.TileContext(nc) as tc:
        build_program(tc, ins, {"y": y.ap()}, meta)
    nc.compile()

    kw = {}
    if trace:
        kw = dict(trace=True, trace_kwargs=trace_kwargs or {})
    res = run_bass_kernel_spmd(nc, in_maps, core_ids=list(range(NCORES)),
                               **kw)
    shards = [r["y"].T for r in res.results]  # each [SHARD, 64]
    out = np.concatenate(shards, axis=0)[:N_NODES]
    return np.ascontiguousarray(out.astype(np.float32)), res


# revision 15
# speedup vs baseline: 1.9742x; 1.0223x over previous
"""CascadeGDCN (3-hop graph diffusion convolution) on 8 Trainium2 NeuronCores.

v4 design:
  - Destination nodes sharded across 8 cores (12544 rows each); edges
    partitioned by destination core; full X replicated per-core in DRAM and
    rebuilt by an AllGather after each hop (skipped after the last hop).
  - Packed-call edge layout: per gather call (4-dest-group block x source
    chunk) the 4 groups' edge segments are packed back-to-back at LANE
    granularity (per-group length = max edge count over the 8 cores, so the
    SPMD program structure is shared); slots of 128 edges may straddle a
    group boundary.  Boundary slots get two one-hot S columns (one per
    group).  This removes most of the per-(group,chunk) ceil padding that a
    slot-aligned layout pays -> ~14% fewer gather descriptors.
  - The gather (Q7 SWDGE descriptor generation at ~2.3 ns/row across 4
    queues) is the kernel bottleneck; idx tables are SBUF-resident and 8
    message buffers keep the gather queues saturated.
  - bf16 compute: S one-hot built by one DVE is_equal per call (vs resident
    denc), edge values folded into messages by one DVE multiply+cast,
    matmuls bf16 (FWL weight load + 1-pass streaming), PSUM accumulates
    [128 dests, 64 feat] per group.
  - st accumulates fp32 in SBUF; new-X rows go psum -> SBUF (ScalarE) ->
    per-group DMA into the bounce buffer feeding the AllGather.
  - The final stage (transpose, Theta matmul, sigmoid, +H) is emitted
    per-block inside the last SpMM so it overlaps the tail of the gathers.
"""

import numpy as np

D = 64
NCORES = 8
NUM_HOPS = 3
N_NODES = 100000
SHARD = 12544            # dest rows per core (98 groups of 128)
NODES_PAD = SHARD * NCORES   # 100352
NCHUNKS = 4
CHUNK = NODES_PAD // NCHUNKS  # 25088 (< 32768 so chunk-local idx fits int16)
GROUPS = SHARD // 128    # 98
GPB = 4                  # dest groups per block (per gather call)
BUFS = 8                 # message-tile pool depth (gather pipelining)


def _softmax(x):
    e = np.exp(x - x.max())
    return e / e.sum()


def _blocks():
    out = []
    g = 0
    while g < GROUPS:
        out.append((g, min(GPB, GROUPS - g)))
        g += GPB
    return out


def _direction_layout(dest, src):
    """Shared (SPMD) packed-call layout for one direction.

    Returns dict with:
      maxc[g, c]      per-cell max edge count over cores
      seg[(g, c)]     lane offset of group g's segment inside call (b, c)
      nreal[(b, c)]   real slots per call;  Oreal[(b, c)] global real offset
      next_[(b, c)]   ext (S) slots per call; Oext[(b, c)] global ext offset
      ents[(b, c)]    list of (slot_local, g, lane_lo, lane_hi) ext entries
      sched[g]        list of (c, ext_local, real_local) matmuls for group g
      nreal_tot, next_tot
    """
    counts = np.zeros((NCORES, GROUPS, NCHUNKS), dtype=np.int64)
    core = dest // SHARD
    for m in range(NCORES):
        sel = core == m
        d_loc = dest[sel] - m * SHARD
        g = d_loc >> 7
        c = src[sel] // CHUNK
        np.add.at(counts, (m, g, c), 1)
    maxc = np.max(counts, axis=0)

    blocks = _blocks()
    seg = {}
    nreal = {}
    next_ = {}
    Oreal = {}
    Oext = {}
    ents = {}
    sched = {g: [] for g in range(GROUPS)}
    pr = 0
    pe = 0
    for bi, (g0, gc) in enumerate(blocks):
        for c in range(NCHUNKS):
            lane = 0
            lo_hi = []
            for gl in range(gc):
                g = g0 + gl
                seg[(g, c)] = lane
                lo_hi.append((g, lane, lane + int(maxc[g, c])))
                lane += int(maxc[g, c])
            ns = (lane + 127) // 128
            nreal[(bi, c)] = ns
            Oreal[(bi, c)] = pr
            pr += ns
            # ext entries: per slot, per overlapping group
            Oext[(bi, c)] = pe
            el = []
            for s in range(ns):
                s_lo, s_hi = s * 128, (s + 1) * 128
                for g, a, b in lo_hi:
                    lo = max(s_lo, a)
                    hi = min(s_hi, b)
                    if lo < hi:
                        el.append((s, g, lo - s_lo, hi - s_lo))
                        sched[g].append((c, len(el) - 1 + pe - Oext[(bi, c)],
                                         s))
            ents[(bi, c)] = el
            next_[(bi, c)] = len(el)
            pe += len(el)
    return {"maxc": maxc, "seg": seg, "nreal": nreal, "next": next_,
            "Oreal": Oreal, "Oext": Oext, "ents": ents, "sched": sched,
            "nreal_tot": pr, "next_tot": pe, "blocks": blocks}


def _prep_direction(dest, src, val, lay):
    """Per-core idx/val (real-slot stream) and denc (ext stream) tables."""
    maxc = lay["maxc"]
    seg = lay["seg"]
    nreal = lay["nreal"]
    Oreal = lay["Oreal"]
    Oext = lay["Oext"]
    ents = lay["ents"]
    blocks = lay["blocks"]
    tot_r = lay["nreal_tot"] * 128
    tot_e = lay["next_tot"]

    # per-cell global lane base = call real base*128 + segment offset
    cell_base = np.zeros(GROUPS * NCHUNKS, dtype=np.int64)
    for g in range(GROUPS):
        bi = g // GPB
        for c in range(NCHUNKS):
            cell_base[g * NCHUNKS + c] = Oreal[(bi, c)] * 128 + seg[(g, c)]

    core = dest // SHARD
    out = []
    for m in range(NCORES):
        sel = core == m
        d_loc = (dest[sel] - m * SHARD).astype(np.int64)
        s = src[sel].astype(np.int64)
        v = val[sel].astype(np.float32)
        g = d_loc >> 7
        c = s // CHUNK
        cell = g * NCHUNKS + c
        order = np.argsort(cell, kind="stable")
        cell_s = cell[order]
        counts = np.bincount(cell_s, minlength=GROUPS * NCHUNKS)
        starts = np.zeros(GROUPS * NCHUNKS, dtype=np.int64)
        starts[1:] = np.cumsum(counts)[:-1]
        rank = np.arange(cell_s.size) - starts[cell_s]
        pos = cell_base[cell_s] + rank

        idx_st = np.zeros(tot_r, dtype=np.int16)
        denc_lane = np.full(tot_r, -1.0, dtype=np.float32)
        val_st = np.zeros(tot_r, dtype=np.float32)
        idx_st[pos] = (s[order] - c[order] * CHUNK).astype(np.int16)
        denc_lane[pos] = (d_loc[order] & 127).astype(np.float32)
        val_st[pos] = v[order]

        # ext denc stream: per ext entry, group lanes only, -1 elsewhere
        denc_ext = np.full((tot_e, 128), -1.0, dtype=np.float32)
        for bi, (g0, gc) in enumerate(blocks):
            for c in range(NCHUNKS):
                ob = Oreal[(bi, c)] * 128
                oe = Oext[(bi, c)]
                for k, (sl, g, lo, hi) in enumerate(ents[(bi, c)]):
                    denc_ext[oe + k, lo:hi] = denc_lane[
                        ob + sl * 128 + lo: ob + sl * 128 + hi]

        idx_tbl = np.tile(np.ascontiguousarray(idx_st.reshape(-1, 16).T),
                          (8, 1))
        denc_tbl = np.ascontiguousarray(denc_ext.T)
        val_tbl = np.ascontiguousarray(val_st.reshape(-1, 128).T)
        out.append({"idx": idx_tbl, "denc": denc_tbl, "val": val_tbl})
    return out


def prep_host(H_l, edge_row, edge_col, edge_val, out_degree, in_degree,
              hop_attention, theta_out, theta_in, Theta):
    from ml_dtypes import bfloat16

    H = np.asarray(H_l, dtype=np.float32)
    er = np.asarray(edge_row, dtype=np.int64)
    ec = np.asarray(edge_col, dtype=np.int64)
    ev = np.asarray(edge_val, dtype=np.float32)
    od = np.asarray(out_degree, dtype=np.float32)
    idg = np.asarray(in_degree, dtype=np.float32)

    alpha = _softmax(np.asarray(hop_attention, dtype=np.float64))
    th_o = np.asarray(theta_out, dtype=np.float64)
    th_i = np.asarray(theta_in, dtype=np.float64)
    coef = [(float(alpha[k] * th_o[k]), float(alpha[k] * th_i[k]))
            for k in range(len(alpha))]

    lay0 = _direction_layout(er, ec)
    lay1 = _direction_layout(ec, er)
    t0 = _prep_direction(er, ec, ev, lay0)
    t1 = _prep_direction(ec, er, ev, lay1)

    x0o = np.zeros((NODES_PAD, D), dtype=np.float32)
    x0i = np.zeros((NODES_PAD, D), dtype=np.float32)
    x0o[:N_NODES] = np.maximum(od, 1e-8)[:, None] * H
    x0i[:N_NODES] = np.maximum(idg, 1e-8)[:, None] * H

    hpad = np.zeros((NODES_PAD, D), dtype=np.float32)
    hpad[:N_NODES] = H
    ident = np.eye(128, dtype=np.float32)
    theta = np.ascontiguousarray(np.asarray(Theta, dtype=np.float32)).astype(
        bfloat16)

    nsmax = 0
    nemax = 0
    for lay in (lay0, lay1):
        nb = {}
        ne = {}
        for (bi, c), v in lay["nreal"].items():
            nb[bi] = nb.get(bi, 0) + v
        for (bi, c), v in lay["next"].items():
            ne[bi] = ne.get(bi, 0) + v
        lay["nblk"] = nb
        lay["neblk"] = ne
        nsmax = max(nsmax, max(nb.values()))
        nemax = max(nemax, max(ne.values()))
    iota = np.tile(np.arange(128, dtype=np.float32), 1)[None, :].repeat(
        128, axis=0).astype(bfloat16)

    in_maps = []
    for m in range(NCORES):
        in_maps.append({
            "x0_out": x0o,
            "x0_in": x0i,
            "hfm": np.ascontiguousarray(hpad[m * SHARD:(m + 1) * SHARD].T),
            "theta": theta,
            "ident": ident,
            "iota": iota,
            "idx0": t0[m]["idx"],
            "denc0": t0[m]["denc"].astype(bfloat16),
            "val0": t0[m]["val"],
            "idx1": t1[m]["idx"],
            "denc1": t1[m]["denc"].astype(bfloat16),
            "val1": t1[m]["val"],
        })
    meta = {"coef": coef, "lay": [lay0, lay1], "nsmax": int(nsmax),
            "nemax": int(nemax)}
    return in_maps, meta


def build_program(tc, ins, outs, meta):
    """Emit the full SPMD program into TileContext tc."""
    import concourse.mybir as mybir

    nc = tc.nc
    f32 = mybir.dt.float32
    f32r = mybir.dt.float32r
    bf16 = mybir.dt.bfloat16
    i16 = mybir.dt.int16
    EQ, MUL, ADD = (mybir.AluOpType.is_equal, mybir.AluOpType.mult,
                    mybir.AluOpType.add)

    coef = meta["coef"]
    nsmax = meta["nsmax"]
    nemax = meta["nemax"]
    lays = meta["lay"]
    rg = [list(range(NCORES))]

    bounce = [nc.dram_tensor(f"bounce{d}", [SHARD, D], f32r,
                             kind="Internal") for d in range(2)]
    xbuf = [[nc.dram_tensor(f"xbuf{d}_{p}", [NODES_PAD, D], f32r,
                            kind="Internal", addr_space="Shared")
             for p in range(2)] for d in range(2)]

    tabs = [
        (ins["idx0"], ins["denc0"], ins["val0"]),
        (ins["idx1"], ins["denc1"], ins["val1"]),
    ]
    x0 = [ins["x0_out"], ins["x0_in"]]

    with (
        tc.tile_pool(name="const", bufs=1) as cpool,
        tc.tile_pool(name="work", bufs=1) as wpool,
        tc.tile_pool(name="stream", bufs=2) as spool,
        tc.tile_pool(name="smat", bufs=2) as spool2,
        tc.tile_pool(name="xc", bufs=4) as xpool,
        tc.tile_pool(name="fin", bufs=1) as fpool,
        tc.tile_pool(name="ps", bufs=4, space="PSUM") as pspool,
        tc.tile_pool(name="psf", bufs=2, space="PSUM") as psfpool,
    ):
        iota_s = cpool.tile([128, 128], bf16, tag="iota")
        nc.sync.dma_start(iota_s[:], ins["iota"][:])
        ident_s = cpool.tile([128, 128], f32, tag="ident")
        nc.sync.dma_start(ident_s[:], ins["ident"][:])
        theta_s = cpool.tile([64, D], bf16, tag="theta")
        nc.sync.dma_start(theta_s[:], ins["theta"][:])

        # resident per-direction idx / denc (bf16, ext) / val (f32) tables
        denc_res = []
        val_res = []
        idx_res = []
        for d in range(2):
            it_ = wpool.tile([128, lays[d]["nreal_tot"] * 8], i16,
                             tag=f"idx{d}")
            nc.sync.dma_start(it_[:], tabs[d][0][:])
            dt_ = wpool.tile([128, lays[d]["next_tot"]], bf16, tag=f"denc{d}")
            nc.sync.dma_start(dt_[:], tabs[d][1][:])
            vt_ = wpool.tile([128, lays[d]["nreal_tot"]], f32, tag=f"val{d}")
            nc.sync.dma_start(vt_[:], tabs[d][2][:])
            idx_res.append(it_)
            denc_res.append(dt_)
            val_res.append(vt_)

        st = wpool.tile([128, GROUPS, D], f32, tag="st")
        nc.vector.memset(st[:], 0.0)

        def emit_final(gs, gcnt):
            width = gcnt * 128
            stfm = fpool.tile([64, GPB * 128], bf16, tag="stfm")
            for j in range(gcnt):
                pt = psfpool.tile([64, 128], f32, tag="pt")
                nc.tensor.transpose(pt[:], st[:, gs + j, :], ident_s[:])
                nc.scalar.copy(out=stfm[:, j * 128:(j + 1) * 128], in_=pt[:])
            zp = psfpool.tile([64, GPB * 128], f32, tag="zp")
            nc.tensor.matmul(zp[:, :width], lhsT=theta_s[:],
                             rhs=stfm[:, :width], start=True, stop=True)
            sg = fpool.tile([64, GPB * 128], f32, tag="sg")
            nc.scalar.activation(sg[:, :width], zp[:, :width],
                                 mybir.ActivationFunctionType.Sigmoid)
            hf = fpool.tile([64, GPB * 128], f32, tag="hf")
            nc.sync.dma_start(
                hf[:, :width], ins["hfm"][:, gs * 128:gs * 128 + width])
            yt = fpool.tile([64, GPB * 128], f32, tag="yt")
            nc.vector.tensor_tensor(out=yt[:, :width], in0=sg[:, :width],
                                    in1=hf[:, :width], op=ADD)
            nc.sync.dma_start(
                outs["y"][:, gs * 128:gs * 128 + width], yt[:, :width])

        for hop in range(NUM_HOPS):
            for dirn in range(2):
                lay = lays[dirn]
                blocks = lay["blocks"]
                sched = lay["sched"]
                xsrc = (x0[dirn] if hop == 0
                        else xbuf[dirn][(hop - 1) % 2].ap())
                xv = xsrc.rearrange("(c n) d -> c n d", n=CHUNK)
                last = hop == NUM_HOPS - 1 and dirn == 1

                for bi, (g0, gc) in enumerate(blocks):
                    nb = lay["nblk"][bi]
                    neb = lay["neblk"][bi]
                    ob = lay["Oreal"][(bi, 0)]
                    oeb = lay["Oext"][(bi, 0)]
                    msgs = spool.tile([128, nsmax, D], f32r, tag="msgs")
                    for c in range(NCHUNKS):
                        ns = lay["nreal"][(bi, c)]
                        if ns == 0:
                            continue
                        o = lay["Oreal"][(bi, c)]
                        ol = o - ob
                        nc.gpsimd.dma_gather(
                            out_ap=msgs[:, ol:ol + ns, :],
                            in_ap=xv[c].bitcast(f32r),
                            idxs_ap=idx_res[dirn][:, o * 8:(o + ns) * 8],
                            num_idxs=ns * 128,
                            num_idxs_reg=ns * 128,
                            elem_size=D,
                            single_packet=False,
                            queue_num=c,
                        )
                    S = spool2.tile([128, nemax, 128], bf16, tag="S")
                    nc.vector.tensor_tensor(
                        out=S[:, :neb, :],
                        in0=iota_s[:].rearrange(
                            "p (o c) -> p o c", c=128).broadcast_to(
                            [128, neb, 128]),
                        in1=denc_res[dirn][:, oeb:oeb + neb].broadcast_to(
                            [128, neb, 128]),
                        op=EQ)
                    mbf = spool2.tile([128, nsmax, D], bf16, tag="mbf")
                    nc.vector.tensor_tensor(
                        out=mbf[:, :nb, :],
                        in0=msgs[:, :nb, :].bitcast(f32),
                        in1=val_res[dirn][:, ob:ob + nb].broadcast_to(
                            [128, nb, D]),
                        op=MUL)

                    for gl in range(gc):
                        g = g0 + gl
                        mms = sched[g]
                        if not mms:
                            continue
                        ps = pspool.tile([128, D], f32, tag="ps")
                        for j, (c, exl, rel) in enumerate(mms):
                            exb = lay["Oext"][(bi, c)] - oeb + exl
                            reb = lay["Oreal"][(bi, c)] - ob + rel
                            nc.tensor.matmul(
                                ps[:],
                                lhsT=S[:, exb, :],
                                rhs=mbf[:, reb, :],
                                start=(j == 0),
                                stop=(j == len(mms) - 1),
                            )
                        nc.vector.scalar_tensor_tensor(
                            out=st[:, g, :], in0=ps[:],
                            scalar=coef[hop][dirn], in1=st[:, g, :],
                            op0=MUL, op1=ADD)
                        if hop < NUM_HOPS - 1:
                            xc = xpool.tile([128, D], f32r, tag="xc")
                            nc.scalar.copy(out=xc[:], in_=ps[:])
                            bounce_v = bounce[dirn].ap().rearrange(
                                "(g p) f -> p g f", p=128)
                            nc.sync.dma_start(bounce_v[:, g, :], xc[:])
                    if last:
                        emit_final(g0, gc)

                if hop < NUM_HOPS - 1:
                    nc.gpsimd.collective_compute(
                        "AllGather", mybir.AluOpType.bypass,
                        replica_groups=rg,
                        ins=[bounce[dirn].ap().opt()],
                        outs=[xbuf[dirn][hop % 2].ap().opt()],
                    )


def kernel(**inputs) -> np.ndarray:
    return _run(inputs, trace=False)[0]


def kernel_traced(inputs, trace_kwargs=None):
    """Returns (output, BassKernelResults) with NTFF trace if available."""
    return _run(inputs, trace=True, trace_kwargs=trace_kwargs or {})


def _run(inputs, trace=False, trace_kwargs=None):
    import concourse.bacc as bacc
    import concourse.mybir as mybir
    import concourse.tile as tile
    from concourse.bass_utils import run_bass_kernel_spmd

    in_maps, meta = prep_host(**inputs)

    nc = bacc.Bacc("TRN2", target_bir_lowering=False, debug=False,
                   num_devices=NCORES, num_swdge_queues=4)
    f32 = mybir.dt.float32
    f32r = mybir.dt.float32r
    bf16 = mybir.dt.bfloat16
    i16 = mybir.dt.int16
    nsmax = meta["nsmax"]
    nemax = meta["nemax"]
    r0 = meta["lay"][0]["nreal_tot"]
    e0 = meta["lay"][0]["next_tot"]
    r1 = meta["lay"][1]["nreal_tot"]
    e1 = meta["lay"][1]["next_tot"]

    ins = {}
    shapes = {
        "x0_out": ([NODES_PAD, D], f32r),
        "x0_in": ([NODES_PAD, D], f32r),
        "hfm": ([D, SHARD], f32),
        "theta": ([D, D], bf16),
        "ident": ([128, 128], f32),
        "iota": ([128, 128], bf16),
        "idx0": ([128, r0 * 8], i16),
        "denc0": ([128, e0], bf16),
        "val0": ([128, r0], f32),
        "idx1": ([128, r1 * 8], i16),
        "denc1": ([128, e1], bf16),
        "val1": ([128, r1], f32),
    }
    for k, (shape, dt) in shapes.items():
        ins[k] = nc.dram_tensor(k, shape, dt, kind="ExternalInput").ap()
    y = nc.dram_tensor("y", [D, SHARD], f32, kind="ExternalOutput")

    with tile.TileContext(nc) as tc:
        build_program(tc, ins, {"y": y.ap()}, meta)
    nc.compile()

    kw = {}
    if trace:
        kw = dict(trace=True, trace_kwargs=trace_kwargs or {})
    res = run_bass_kernel_spmd(nc, in_maps, core_ids=list(range(NCORES)),
                               **kw)
    shards = [r["y"].T for r in res.results]  # each [SHARD, 64]
    out = np.concatenate(shards, axis=0)[:N_NODES]
    return np.ascontiguousarray(out.astype(np.float32)), res


# revision 16
# speedup vs baseline: 1.9745x; 1.0002x over previous
"""CascadeGDCN (3-hop graph diffusion convolution) on 8 Trainium2 NeuronCores.

v4 design:
  - Destination nodes sharded across 8 cores (12544 rows each); edges
    partitioned by destination core; full X replicated per-core in DRAM and
    rebuilt by an AllGather after each hop (skipped after the last hop).
  - Packed-call edge layout: per gather call (4-dest-group block x source
    chunk) the 4 groups' edge segments are packed back-to-back at LANE
    granularity (per-group length = max edge count over the 8 cores, so the
    SPMD program structure is shared); slots of 128 edges may straddle a
    group boundary.  Boundary slots get two one-hot S columns (one per
    group).  This removes most of the per-(group,chunk) ceil padding that a
    slot-aligned layout pays -> ~14% fewer gather descriptors.
  - The gather (Q7 SWDGE descriptor generation at ~2.3 ns/row across 4
    queues) is the kernel bottleneck; idx tables are SBUF-resident and 8
    message buffers keep the gather queues saturated.
  - bf16 compute: S one-hot built by one DVE is_equal per call (vs resident
    denc), edge values folded into messages by one DVE multiply+cast,
    matmuls bf16 (FWL weight load + 1-pass streaming), PSUM accumulates
    [128 dests, 64 feat] per group.
  - st accumulates fp32 in SBUF; new-X rows go psum -> SBUF (ScalarE) ->
    per-group DMA into the bounce buffer feeding the AllGather.
  - The final stage (transpose, Theta matmul, sigmoid, +H) is emitted
    per-block inside the last SpMM so it overlaps the tail of the gathers.
"""

import numpy as np

D = 64
NCORES = 8
NUM_HOPS = 3
N_NODES = 100000
SHARD = 12544            # dest rows per core (98 groups of 128)
NODES_PAD = SHARD * NCORES   # 100352
NCHUNKS = 4
CHUNK = NODES_PAD // NCHUNKS  # 25088 (< 32768 so chunk-local idx fits int16)
GROUPS = SHARD // 128    # 98
GPB = 4                  # dest groups per block (4 gather calls per block)


def _softmax(x):
    e = np.exp(x - x.max())
    return e / e.sum()


def _blocks():
    out = []
    g = 0
    while g < GROUPS:
        out.append((g, min(GPB, GROUPS - g)))
        g += GPB
    return out


def _direction_layout(dest, src):
    """Shared (SPMD) packed-call layout for one direction.

    Returns dict with:
      maxc[g, c]      per-cell max edge count over cores
      seg[(g, c)]     lane offset of group g's segment inside call (b, c)
      nreal[(b, c)]   real slots per call;  Oreal[(b, c)] global real offset
      next_[(b, c)]   ext (S) slots per call; Oext[(b, c)] global ext offset
      ents[(b, c)]    list of (slot_local, g, lane_lo, lane_hi) ext entries
      sched[g]        list of (c, ext_local, real_local) matmuls for group g
      nreal_tot, next_tot
    """
    counts = np.zeros((NCORES, GROUPS, NCHUNKS), dtype=np.int64)
    core = dest // SHARD
    for m in range(NCORES):
        sel = core == m
        d_loc = dest[sel] - m * SHARD
        g = d_loc >> 7
        c = src[sel] // CHUNK
        np.add.at(counts, (m, g, c), 1)
    maxc = np.max(counts, axis=0)

    blocks = _blocks()
    seg = {}
    nreal = {}
    next_ = {}
    Oreal = {}
    Oext = {}
    ents = {}
    sched = {g: [] for g in range(GROUPS)}
    pr = 0
    pe = 0
    for bi, (g0, gc) in enumerate(blocks):
        for c in range(NCHUNKS):
            lane = 0
            lo_hi = []
            for gl in range(gc):
                g = g0 + gl
                seg[(g, c)] = lane
                lo_hi.append((g, lane, lane + int(maxc[g, c])))
                lane += int(maxc[g, c])
            ns = (lane + 127) // 128
            nreal[(bi, c)] = ns
            Oreal[(bi, c)] = pr
            pr += ns
            # ext entries: per slot, per overlapping group
            Oext[(bi, c)] = pe
            el = []
            for s in range(ns):
                s_lo, s_hi = s * 128, (s + 1) * 128
                for g, a, b in lo_hi:
                    lo = max(s_lo, a)
                    hi = min(s_hi, b)
                    if lo < hi:
                        el.append((s, g, lo - s_lo, hi - s_lo))
                        sched[g].append((c, len(el) - 1 + pe - Oext[(bi, c)],
                                         s))
            ents[(bi, c)] = el
            next_[(bi, c)] = len(el)
            pe += len(el)
    return {"maxc": maxc, "seg": seg, "nreal": nreal, "next": next_,
            "Oreal": Oreal, "Oext": Oext, "ents": ents, "sched": sched,
            "nreal_tot": pr, "next_tot": pe, "blocks": blocks}


def _prep_direction(dest, src, val, lay):
    """Per-core idx/val (real-slot stream) and denc (ext stream) tables."""
    maxc = lay["maxc"]
    seg = lay["seg"]
    nreal = lay["nreal"]
    Oreal = lay["Oreal"]
    Oext = lay["Oext"]
    ents = lay["ents"]
    blocks = lay["blocks"]
    tot_r = lay["nreal_tot"] * 128
    tot_e = lay["next_tot"]

    # per-cell global lane base = call real base*128 + segment offset
    cell_base = np.zeros(GROUPS * NCHUNKS, dtype=np.int64)
    for g in range(GROUPS):
        bi = g // GPB
        for c in range(NCHUNKS):
            cell_base[g * NCHUNKS + c] = Oreal[(bi, c)] * 128 + seg[(g, c)]

    core = dest // SHARD
    out = []
    for m in range(NCORES):
        sel = core == m
        d_loc = (dest[sel] - m * SHARD).astype(np.int64)
        s = src[sel].astype(np.int64)
        v = val[sel].astype(np.float32)
        g = d_loc >> 7
        c = s // CHUNK
        cell = g * NCHUNKS + c
        order = np.argsort(cell, kind="stable")
        cell_s = cell[order]
        counts = np.bincount(cell_s, minlength=GROUPS * NCHUNKS)
        starts = np.zeros(GROUPS * NCHUNKS, dtype=np.int64)
        starts[1:] = np.cumsum(counts)[:-1]
        rank = np.arange(cell_s.size) - starts[cell_s]
        pos = cell_base[cell_s] + rank

        idx_st = np.zeros(tot_r, dtype=np.int16)
        denc_lane = np.full(tot_r, -1.0, dtype=np.float32)
        val_st = np.zeros(tot_r, dtype=np.float32)
        idx_st[pos] = (s[order] - c[order] * CHUNK).astype(np.int16)
        denc_lane[pos] = (d_loc[order] & 127).astype(np.float32)
        val_st[pos] = v[order]

        # ext denc stream: per ext entry, group lanes only, -1 elsewhere
        denc_ext = np.full((tot_e, 128), -1.0, dtype=np.float32)
        for bi, (g0, gc) in enumerate(blocks):
            for c in range(NCHUNKS):
                ob = Oreal[(bi, c)] * 128
                oe = Oext[(bi, c)]
                for k, (sl, g, lo, hi) in enumerate(ents[(bi, c)]):
                    denc_ext[oe + k, lo:hi] = denc_lane[
                        ob + sl * 128 + lo: ob + sl * 128 + hi]

        idx_tbl = np.tile(np.ascontiguousarray(idx_st.reshape(-1, 16).T),
                          (8, 1))
        denc_tbl = np.ascontiguousarray(denc_ext.T)
        val_tbl = np.ascontiguousarray(val_st.reshape(-1, 128).T)
        out.append({"idx": idx_tbl, "denc": denc_tbl, "val": val_tbl})
    return out


def prep_host(H_l, edge_row, edge_col, edge_val, out_degree, in_degree,
              hop_attention, theta_out, theta_in, Theta):
    from ml_dtypes import bfloat16

    H = np.asarray(H_l, dtype=np.float32)
    er = np.asarray(edge_row, dtype=np.int64)
    ec = np.asarray(edge_col, dtype=np.int64)
    ev = np.asarray(edge_val, dtype=np.float32)
    od = np.asarray(out_degree, dtype=np.float32)
    idg = np.asarray(in_degree, dtype=np.float32)

    alpha = _softmax(np.asarray(hop_attention, dtype=np.float64))
    th_o = np.asarray(theta_out, dtype=np.float64)
    th_i = np.asarray(theta_in, dtype=np.float64)
    coef = [(float(alpha[k] * th_o[k]), float(alpha[k] * th_i[k]))
            for k in range(len(alpha))]

    lay0 = _direction_layout(er, ec)
    lay1 = _direction_layout(ec, er)
    t0 = _prep_direction(er, ec, ev, lay0)
    t1 = _prep_direction(ec, er, ev, lay1)

    x0o = np.zeros((NODES_PAD, D), dtype=np.float32)
    x0i = np.zeros((NODES_PAD, D), dtype=np.float32)
    x0o[:N_NODES] = np.maximum(od, 1e-8)[:, None] * H
    x0i[:N_NODES] = np.maximum(idg, 1e-8)[:, None] * H

    hpad = np.zeros((NODES_PAD, D), dtype=np.float32)
    hpad[:N_NODES] = H
    ident = np.eye(128, dtype=np.float32)
    theta = np.ascontiguousarray(np.asarray(Theta, dtype=np.float32)).astype(
        bfloat16)

    nsmax = 0
    nemax = 0
    for lay in (lay0, lay1):
        nb = {}
        ne = {}
        for (bi, c), v in lay["nreal"].items():
            nb[bi] = nb.get(bi, 0) + v
        for (bi, c), v in lay["next"].items():
            ne[bi] = ne.get(bi, 0) + v
        lay["nblk"] = nb
        lay["neblk"] = ne
        nsmax = max(nsmax, max(nb.values()))
        nemax = max(nemax, max(ne.values()))
    iota = np.tile(np.arange(128, dtype=np.float32), 1)[None, :].repeat(
        128, axis=0).astype(bfloat16)

    in_maps = []
    for m in range(NCORES):
        in_maps.append({
            "x0_out": x0o,
            "x0_in": x0i,
            "hfm": np.ascontiguousarray(hpad[m * SHARD:(m + 1) * SHARD].T),
            "theta": theta,
            "ident": ident,
            "iota": iota,
            "idx0": t0[m]["idx"],
            "denc0": t0[m]["denc"].astype(bfloat16),
            "val0": t0[m]["val"],
            "idx1": t1[m]["idx"],
            "denc1": t1[m]["denc"].astype(bfloat16),
            "val1": t1[m]["val"],
        })
    meta = {"coef": coef, "lay": [lay0, lay1], "nsmax": int(nsmax),
            "nemax": int(nemax)}
    return in_maps, meta


def build_program(tc, ins, outs, meta):
    """Emit the full SPMD program into TileContext tc."""
    import concourse.mybir as mybir

    nc = tc.nc
    f32 = mybir.dt.float32
    f32r = mybir.dt.float32r
    bf16 = mybir.dt.bfloat16
    i16 = mybir.dt.int16
    EQ, MUL, ADD = (mybir.AluOpType.is_equal, mybir.AluOpType.mult,
                    mybir.AluOpType.add)

    coef = meta["coef"]
    nsmax = meta["nsmax"]
    nemax = meta["nemax"]
    lays = meta["lay"]
    rg = [list(range(NCORES))]

    bounce = [nc.dram_tensor(f"bounce{d}", [SHARD, D], f32r,
                             kind="Internal") for d in range(2)]
    xbuf = [[nc.dram_tensor(f"xbuf{d}_{p}", [NODES_PAD, D], f32r,
                            kind="Internal", addr_space="Shared")
             for p in range(2)] for d in range(2)]

    tabs = [
        (ins["idx0"], ins["denc0"], ins["val0"]),
        (ins["idx1"], ins["denc1"], ins["val1"]),
    ]
    x0 = [ins["x0_out"], ins["x0_in"]]

    with (
        tc.tile_pool(name="const", bufs=1) as cpool,
        tc.tile_pool(name="work", bufs=1) as wpool,
        tc.tile_pool(name="stream", bufs=2) as spool,
        tc.tile_pool(name="smat", bufs=2) as spool2,
        tc.tile_pool(name="xc", bufs=4) as xpool,
        tc.tile_pool(name="fin", bufs=1) as fpool,
        tc.tile_pool(name="ps", bufs=4, space="PSUM") as pspool,
        tc.tile_pool(name="psf", bufs=2, space="PSUM") as psfpool,
    ):
        iota_s = cpool.tile([128, 128], bf16, tag="iota")
        nc.sync.dma_start(iota_s[:], ins["iota"][:])
        ident_s = cpool.tile([128, 128], f32, tag="ident")
        nc.sync.dma_start(ident_s[:], ins["ident"][:])
        theta_s = cpool.tile([64, D], bf16, tag="theta")
        nc.sync.dma_start(theta_s[:], ins["theta"][:])

        # resident per-direction idx / denc (bf16, ext) / val (f32) tables
        denc_res = []
        val_res = []
        idx_res = []
        for d in range(2):
            it_ = wpool.tile([128, lays[d]["nreal_tot"] * 8], i16,
                             tag=f"idx{d}")
            nc.sync.dma_start(it_[:], tabs[d][0][:])
            dt_ = wpool.tile([128, lays[d]["next_tot"]], bf16, tag=f"denc{d}")
            nc.sync.dma_start(dt_[:], tabs[d][1][:])
            vt_ = wpool.tile([128, lays[d]["nreal_tot"]], f32, tag=f"val{d}")
            nc.sync.dma_start(vt_[:], tabs[d][2][:])
            idx_res.append(it_)
            denc_res.append(dt_)
            val_res.append(vt_)

        st = wpool.tile([128, GROUPS, D], f32, tag="st")
        nc.vector.memset(st[:], 0.0)

        def emit_final(gs, gcnt):
            width = gcnt * 128
            stfm = fpool.tile([64, GPB * 128], bf16, tag="stfm")
            for j in range(gcnt):
                pt = psfpool.tile([64, 128], f32, tag="pt")
                nc.tensor.transpose(pt[:], st[:, gs + j, :], ident_s[:])
                nc.scalar.copy(out=stfm[:, j * 128:(j + 1) * 128], in_=pt[:])
            zp = psfpool.tile([64, GPB * 128], f32, tag="zp")
            nc.tensor.matmul(zp[:, :width], lhsT=theta_s[:],
                             rhs=stfm[:, :width], start=True, stop=True)
            sg = fpool.tile([64, GPB * 128], f32, tag="sg")
            nc.scalar.activation(sg[:, :width], zp[:, :width],
                                 mybir.ActivationFunctionType.Sigmoid)
            hf = fpool.tile([64, GPB * 128], f32, tag="hf")
            nc.sync.dma_start(
                hf[:, :width], ins["hfm"][:, gs * 128:gs * 128 + width])
            yt = fpool.tile([64, GPB * 128], f32, tag="yt")
            nc.vector.tensor_tensor(out=yt[:, :width], in0=sg[:, :width],
                                    in1=hf[:, :width], op=ADD)
            nc.sync.dma_start(
                outs["y"][:, gs * 128:gs * 128 + width], yt[:, :width])

        for hop in range(NUM_HOPS):
            for dirn in range(2):
                lay = lays[dirn]
                blocks = lay["blocks"]
                sched = lay["sched"]
                xsrc = (x0[dirn] if hop == 0
                        else xbuf[dirn][(hop - 1) % 2].ap())
                xv = xsrc.rearrange("(c n) d -> c n d", n=CHUNK)
                last = hop == NUM_HOPS - 1 and dirn == 1

                for bi, (g0, gc) in enumerate(blocks):
                    nb = lay["nblk"][bi]
                    neb = lay["neblk"][bi]
                    ob = lay["Oreal"][(bi, 0)]
                    oeb = lay["Oext"][(bi, 0)]
                    msgs = spool.tile([128, nsmax, D], f32r, tag="msgs")
                    for c in range(NCHUNKS):
                        ns = lay["nreal"][(bi, c)]
                        if ns == 0:
                            continue
                        o = lay["Oreal"][(bi, c)]
                        ol = o - ob
                        nc.gpsimd.dma_gather(
                            out_ap=msgs[:, ol:ol + ns, :],
                            in_ap=xv[c].bitcast(f32r),
                            idxs_ap=idx_res[dirn][:, o * 8:(o + ns) * 8],
                            num_idxs=ns * 128,
                            num_idxs_reg=ns * 128,
                            elem_size=D,
                            single_packet=False,
                            queue_num=c,
                        )
                    S = spool2.tile([128, nemax, 128], bf16, tag="S")
                    nc.vector.tensor_tensor(
                        out=S[:, :neb, :],
                        in0=iota_s[:].rearrange(
                            "p (o c) -> p o c", c=128).broadcast_to(
                            [128, neb, 128]),
                        in1=denc_res[dirn][:, oeb:oeb + neb].broadcast_to(
                            [128, neb, 128]),
                        op=EQ)
                    mbf = spool2.tile([128, nsmax, D], bf16, tag="mbf")
                    nc.vector.tensor_tensor(
                        out=mbf[:, :nb, :],
                        in0=msgs[:, :nb, :].bitcast(f32),
                        in1=val_res[dirn][:, ob:ob + nb].broadcast_to(
                            [128, nb, D]),
                        op=MUL)

                    for gl in range(gc):
                        g = g0 + gl
                        mms = sched[g]
                        if not mms:
                            continue
                        ps = pspool.tile([128, D], f32, tag="ps")
                        for j, (c, exl, rel) in enumerate(mms):
                            exb = lay["Oext"][(bi, c)] - oeb + exl
                            reb = lay["Oreal"][(bi, c)] - ob + rel
                            nc.tensor.matmul(
                                ps[:],
                                lhsT=S[:, exb, :],
                                rhs=mbf[:, reb, :],
                                start=(j == 0),
                                stop=(j == len(mms) - 1),
                            )
                        nc.vector.scalar_tensor_tensor(
                            out=st[:, g, :], in0=ps[:],
                            scalar=coef[hop][dirn], in1=st[:, g, :],
                            op0=MUL, op1=ADD)
                        if hop < NUM_HOPS - 1:
                            xc = xpool.tile([128, D], f32r, tag="xc")
                            nc.scalar.copy(out=xc[:], in_=ps[:])
                            bounce_v = bounce[dirn].ap().rearrange(
                                "(g p) f -> p g f", p=128)
                            nc.sync.dma_start(bounce_v[:, g, :], xc[:])
                    if last:
                        emit_final(g0, gc)

                if hop < NUM_HOPS - 1:
                    nc.gpsimd.collective_compute(
                        "AllGather", mybir.AluOpType.bypass,
                        replica_groups=rg,
                        ins=[bounce[dirn].ap().opt()],
                        outs=[xbuf[dirn][hop % 2].ap().opt()],
                    )


def kernel(**inputs) -> np.ndarray:
    return _run(inputs, trace=False)[0]


def kernel_traced(inputs, trace_kwargs=None):
    """Returns (output, BassKernelResults) with NTFF trace if available."""
    return _run(inputs, trace=True, trace_kwargs=trace_kwargs or {})


def _run(inputs, trace=False, trace_kwargs=None):
    import concourse.bacc as bacc
    import concourse.mybir as mybir
    import concourse.tile as tile
    from concourse.bass_utils import run_bass_kernel_spmd

    in_maps, meta = prep_host(**inputs)

    nc = bacc.Bacc("TRN2", target_bir_lowering=False, debug=False,
                   num_devices=NCORES, num_swdge_queues=4)
    f32 = mybir.dt.float32
    f32r = mybir.dt.float32r
    bf16 = mybir.dt.bfloat16
    i16 = mybir.dt.int16
    nsmax = meta["nsmax"]
    nemax = meta["nemax"]
    r0 = meta["lay"][0]["nreal_tot"]
    e0 = meta["lay"][0]["next_tot"]
    r1 = meta["lay"][1]["nreal_tot"]
    e1 = meta["lay"][1]["next_tot"]

    ins = {}
    shapes = {
        "x0_out": ([NODES_PAD, D], f32r),
        "x0_in": ([NODES_PAD, D], f32r),
        "hfm": ([D, SHARD], f32),
        "theta": ([D, D], bf16),
        "ident": ([128, 128], f32),
        "iota": ([128, 128], bf16),
        "idx0": ([128, r0 * 8], i16),
        "denc0": ([128, e0], bf16),
        "val0": ([128, r0], f32),
        "idx1": ([128, r1 * 8], i16),
        "denc1": ([128, e1], bf16),
        "val1": ([128, r1], f32),
    }
    for k, (shape, dt) in shapes.items():
        ins[k] = nc.dram_tensor(k, shape, dt, kind="ExternalInput").ap()
    y = nc.dram_tensor("y", [D, SHARD], f32, kind="ExternalOutput")

    with tile.TileContext(nc) as tc:
        build_program(tc, ins, {"y": y.ap()}, meta)
    nc.compile()

    kw = {}
    if trace:
        kw = dict(trace=True, trace_kwargs=trace_kwargs or {})
    res = run_bass_kernel_spmd(nc, in_maps, core_ids=list(range(NCORES)),
                               **kw)
    shards = [r["y"].T for r in res.results]  # each [SHARD, 64]
    out = np.concatenate(shards, axis=0)[:N_NODES]
    return np.ascontiguousarray(out.astype(np.float32)), res


# revision 21
# speedup vs baseline: 2.0105x; 1.0182x over previous
"""CascadeGDCN (3-hop graph diffusion convolution) on 8 Trainium2 NeuronCores.

v4 design:
  - Destination nodes sharded across 8 cores (12544 rows each); edges
    partitioned by destination core; full X replicated per-core in DRAM and
    rebuilt by an AllGather after each hop (skipped after the last hop).
  - Packed-call edge layout: per gather call (4-dest-group block x source
    chunk) the 4 groups' edge segments are packed back-to-back at LANE
    granularity (per-group length = max edge count over the 8 cores, so the
    SPMD program structure is shared); slots of 128 edges may straddle a
    group boundary.  Boundary slots get two one-hot S columns (one per
    group).  This removes most of the per-(group,chunk) ceil padding that a
    slot-aligned layout pays -> ~14% fewer gather descriptors.
  - The gather (Q7 SWDGE descriptor generation at ~2.3 ns/row across 4
    queues) is the kernel bottleneck; idx tables are SBUF-resident and 8
    message buffers keep the gather queues saturated.
  - bf16 compute: S one-hot built by one DVE is_equal per call (vs resident
    denc), edge values folded into messages by one DVE multiply+cast,
    matmuls bf16 (FWL weight load + 1-pass streaming), PSUM accumulates
    [128 dests, 64 feat] per group.
  - st accumulates fp32 in SBUF; new-X rows go psum -> SBUF (ScalarE) ->
    per-group DMA into the bounce buffer feeding the AllGather.
  - The final stage (transpose, Theta matmul, sigmoid, +H) is emitted
    per-block inside the last SpMM so it overlaps the tail of the gathers.
"""

import numpy as np

D = 64
NCORES = 8
NUM_HOPS = 3
N_NODES = 100000
SHARD = 12544            # dest rows per core (98 groups of 128)
NODES_PAD = SHARD * NCORES   # 100352
NCHUNKS = 4
CHUNK = NODES_PAD // NCHUNKS  # 25088 (< 32768 so chunk-local idx fits int16)
GROUPS = SHARD // 128    # 98
GPB = 4                  # dest groups per block (per gather call)
BUFS = 8                 # message-tile pool depth (gather pipelining)


def _softmax(x):
    e = np.exp(x - x.max())
    return e / e.sum()


def _blocks():
    out = []
    g = 0
    while g < GROUPS:
        out.append((g, min(GPB, GROUPS - g)))
        g += GPB
    return out


def _direction_layout(dest, src):
    """Shared (SPMD) packed-call layout for one direction.

    Returns dict with:
      maxc[g, c]      per-cell max edge count over cores
      seg[(g, c)]     lane offset of group g's segment inside call (b, c)
      nreal[(b, c)]   real slots per call;  Oreal[(b, c)] global real offset
      next_[(b, c)]   ext (S) slots per call; Oext[(b, c)] global ext offset
      ents[(b, c)]    list of (slot_local, g, lane_lo, lane_hi) ext entries
      sched[g]        list of (c, ext_local, real_local) matmuls for group g
      nreal_tot, next_tot
    """
    counts = np.zeros((NCORES, GROUPS, NCHUNKS), dtype=np.int64)
    core = dest // SHARD
    for m in range(NCORES):
        sel = core == m
        d_loc = dest[sel] - m * SHARD
        g = d_loc >> 7
        c = src[sel] // CHUNK
        np.add.at(counts, (m, g, c), 1)
    maxc = np.max(counts, axis=0)

    blocks = _blocks()
    seg = {}
    nreal = {}
    next_ = {}
    Oreal = {}
    Oext = {}
    ents = {}
    sched = {g: [] for g in range(GROUPS)}
    pr = 0
    pe = 0
    for bi, (g0, gc) in enumerate(blocks):
        for c in range(NCHUNKS):
            lane = 0
            lo_hi = []
            for gl in range(gc):
                g = g0 + gl
                seg[(g, c)] = lane
                lo_hi.append((g, lane, lane + int(maxc[g, c])))
                lane += int(maxc[g, c])
            ns = (lane + 127) // 128
            nreal[(bi, c)] = ns
            Oreal[(bi, c)] = pr
            pr += ns
            # ext entries: per slot, per overlapping group
            Oext[(bi, c)] = pe
            el = []
            for s in range(ns):
                s_lo, s_hi = s * 128, (s + 1) * 128
                for g, a, b in lo_hi:
                    lo = max(s_lo, a)
                    hi = min(s_hi, b)
                    if lo < hi:
                        el.append((s, g, lo - s_lo, hi - s_lo))
                        sched[g].append((c, len(el) - 1 + pe - Oext[(bi, c)],
                                         s))
            ents[(bi, c)] = el
            next_[(bi, c)] = len(el)
            pe += len(el)
    return {"maxc": maxc, "seg": seg, "nreal": nreal, "next": next_,
            "Oreal": Oreal, "Oext": Oext, "ents": ents, "sched": sched,
            "nreal_tot": pr, "next_tot": pe, "blocks": blocks}


def _prep_direction(dest, src, val, lay):
    """Per-core idx/val (real-slot stream) and denc (ext stream) tables."""
    maxc = lay["maxc"]
    seg = lay["seg"]
    nreal = lay["nreal"]
    Oreal = lay["Oreal"]
    Oext = lay["Oext"]
    ents = lay["ents"]
    blocks = lay["blocks"]
    tot_r = lay["nreal_tot"] * 128
    tot_e = lay["next_tot"]

    # per-cell global lane base = call real base*128 + segment offset
    cell_base = np.zeros(GROUPS * NCHUNKS, dtype=np.int64)
    for g in range(GROUPS):
        bi = g // GPB
        for c in range(NCHUNKS):
            cell_base[g * NCHUNKS + c] = Oreal[(bi, c)] * 128 + seg[(g, c)]

    core = dest // SHARD
    out = []
    for m in range(NCORES):
        sel = core == m
        d_loc = (dest[sel] - m * SHARD).astype(np.int64)
        s = src[sel].astype(np.int64)
        v = val[sel].astype(np.float32)
        g = d_loc >> 7
        c = s // CHUNK
        cell = g * NCHUNKS + c
        order = np.argsort(cell, kind="stable")
        cell_s = cell[order]
        counts = np.bincount(cell_s, minlength=GROUPS * NCHUNKS)
        starts = np.zeros(GROUPS * NCHUNKS, dtype=np.int64)
        starts[1:] = np.cumsum(counts)[:-1]
        rank = np.arange(cell_s.size) - starts[cell_s]
        pos = cell_base[cell_s] + rank

        idx_st = np.zeros(tot_r, dtype=np.int16)
        denc_lane = np.full(tot_r, -1.0, dtype=np.float32)
        val_st = np.zeros(tot_r, dtype=np.float32)
        idx_st[pos] = (s[order] - c[order] * CHUNK).astype(np.int16)
        denc_lane[pos] = (d_loc[order] & 127).astype(np.float32)
        val_st[pos] = v[order]

        # ext denc stream: per ext entry, group lanes only, -1 elsewhere
        denc_ext = np.full((tot_e, 128), -1.0, dtype=np.float32)
        for bi, (g0, gc) in enumerate(blocks):
            for c in range(NCHUNKS):
                ob = Oreal[(bi, c)] * 128
                oe = Oext[(bi, c)]
                for k, (sl, g, lo, hi) in enumerate(ents[(bi, c)]):
                    denc_ext[oe + k, lo:hi] = denc_lane[
                        ob + sl * 128 + lo: ob + sl * 128 + hi]

        idx_tbl = np.tile(np.ascontiguousarray(idx_st.reshape(-1, 16).T),
                          (8, 1))
        denc_tbl = np.ascontiguousarray(denc_ext.T)
        val_tbl = np.ascontiguousarray(val_st.reshape(-1, 128).T)
        out.append({"idx": idx_tbl, "denc": denc_tbl, "val": val_tbl})
    return out


def prep_host(H_l, edge_row, edge_col, edge_val, out_degree, in_degree,
              hop_attention, theta_out, theta_in, Theta):
    from ml_dtypes import bfloat16

    H = np.asarray(H_l, dtype=np.float32)
    er = np.asarray(edge_row, dtype=np.int64)
    ec = np.asarray(edge_col, dtype=np.int64)
    ev = np.asarray(edge_val, dtype=np.float32)
    od = np.asarray(out_degree, dtype=np.float32)
    idg = np.asarray(in_degree, dtype=np.float32)

    alpha = _softmax(np.asarray(hop_attention, dtype=np.float64))
    th_o = np.asarray(theta_out, dtype=np.float64)
    th_i = np.asarray(theta_in, dtype=np.float64)
    coef = [(float(alpha[k] * th_o[k]), float(alpha[k] * th_i[k]))
            for k in range(len(alpha))]

    lay0 = _direction_layout(er, ec)
    lay1 = _direction_layout(ec, er)
    t0 = _prep_direction(er, ec, ev, lay0)
    t1 = _prep_direction(ec, er, ev, lay1)

    x0o = np.zeros((NODES_PAD, D), dtype=np.float32)
    x0i = np.zeros((NODES_PAD, D), dtype=np.float32)
    x0o[:N_NODES] = np.maximum(od, 1e-8)[:, None] * H
    x0i[:N_NODES] = np.maximum(idg, 1e-8)[:, None] * H

    hpad = np.zeros((NODES_PAD, D), dtype=np.float32)
    hpad[:N_NODES] = H
    ident = np.eye(128, dtype=np.float32)
    theta = np.ascontiguousarray(np.asarray(Theta, dtype=np.float32)).astype(
        bfloat16)

    nsmax = 0
    nemax = 0
    for lay in (lay0, lay1):
        nb = {}
        ne = {}
        for (bi, c), v in lay["nreal"].items():
            nb[bi] = nb.get(bi, 0) + v
        for (bi, c), v in lay["next"].items():
            ne[bi] = ne.get(bi, 0) + v
        lay["nblk"] = nb
        lay["neblk"] = ne
        nsmax = max(nsmax, max(nb.values()))
        nemax = max(nemax, max(ne.values()))
    iota = np.tile(np.arange(128, dtype=np.float32), 1)[None, :].repeat(
        128, axis=0).astype(bfloat16)

    in_maps = []
    for m in range(NCORES):
        in_maps.append({
            "x0_out": x0o,
            "x0_in": x0i,
            "hfm": np.ascontiguousarray(hpad[m * SHARD:(m + 1) * SHARD].T),
            "theta": theta,
            "ident": ident,
            "iota": iota,
            "idx0": t0[m]["idx"],
            "denc0": t0[m]["denc"].astype(bfloat16),
            "val0": t0[m]["val"],
            "idx1": t1[m]["idx"],
            "denc1": t1[m]["denc"].astype(bfloat16),
            "val1": t1[m]["val"],
        })
    meta = {"coef": coef, "lay": [lay0, lay1], "nsmax": int(nsmax),
            "nemax": int(nemax)}
    return in_maps, meta


def build_program(tc, ins, outs, meta):
    """Emit the full SPMD program into TileContext tc."""
    import concourse.mybir as mybir

    nc = tc.nc
    f32 = mybir.dt.float32
    f32r = mybir.dt.float32r
    bf16 = mybir.dt.bfloat16
    i16 = mybir.dt.int16
    EQ, MUL, ADD = (mybir.AluOpType.is_equal, mybir.AluOpType.mult,
                    mybir.AluOpType.add)

    coef = meta["coef"]
    nsmax = meta["nsmax"]
    nemax = meta["nemax"]
    lays = meta["lay"]
    rg = [list(range(NCORES))]

    bounce = [nc.dram_tensor(f"bounce{d}", [SHARD, D], f32r,
                             kind="Internal") for d in range(2)]
    xbuf = [[nc.dram_tensor(f"xbuf{d}_{p}", [NODES_PAD, D], f32r,
                            kind="Internal", addr_space="Shared")
             for p in range(2)] for d in range(2)]

    tabs = [
        (ins["idx0"], ins["denc0"], ins["val0"]),
        (ins["idx1"], ins["denc1"], ins["val1"]),
    ]
    x0 = [ins["x0_out"], ins["x0_in"]]

    with (
        tc.tile_pool(name="const", bufs=1) as cpool,
        tc.tile_pool(name="work", bufs=1) as wpool,
        tc.tile_pool(name="stream", bufs=2) as spool,
        tc.tile_pool(name="smat", bufs=2) as spool2,
        tc.tile_pool(name="xc", bufs=4) as xpool,
        tc.tile_pool(name="fin", bufs=1) as fpool,
        tc.tile_pool(name="ps", bufs=4, space="PSUM") as pspool,
        tc.tile_pool(name="psf", bufs=2, space="PSUM") as psfpool,
    ):
        iota_s = cpool.tile([128, 128], bf16, tag="iota")
        nc.sync.dma_start(iota_s[:], ins["iota"][:])
        ident_s = cpool.tile([128, 128], f32, tag="ident")
        nc.sync.dma_start(ident_s[:], ins["ident"][:])
        theta_s = cpool.tile([64, D], bf16, tag="theta")
        nc.sync.dma_start(theta_s[:], ins["theta"][:])

        # resident per-direction idx / denc (bf16, ext) / val (f32) tables
        denc_res = []
        val_res = []
        idx_res = []
        for d in range(2):
            it_ = wpool.tile([128, lays[d]["nreal_tot"] * 8], i16,
                             tag=f"idx{d}")
            nc.sync.dma_start(it_[:], tabs[d][0][:])
            dt_ = wpool.tile([128, lays[d]["next_tot"]], bf16, tag=f"denc{d}")
            nc.sync.dma_start(dt_[:], tabs[d][1][:])
            vt_ = wpool.tile([128, lays[d]["nreal_tot"]], f32, tag=f"val{d}")
            nc.sync.dma_start(vt_[:], tabs[d][2][:])
            idx_res.append(it_)
            denc_res.append(dt_)
            val_res.append(vt_)

        st = wpool.tile([128, GROUPS, D], f32, tag="st")
        nc.vector.memset(st[:], 0.0)

        def emit_final(gs, gcnt):
            width = gcnt * 128
            stfm = fpool.tile([64, GPB * 128], bf16, tag="stfm")
            for j in range(gcnt):
                pt = psfpool.tile([64, 128], f32, tag="pt")
                nc.tensor.transpose(pt[:], st[:, gs + j, :], ident_s[:])
                nc.scalar.copy(out=stfm[:, j * 128:(j + 1) * 128], in_=pt[:])
            zp = psfpool.tile([64, GPB * 128], f32, tag="zp")
            nc.tensor.matmul(zp[:, :width], lhsT=theta_s[:],
                             rhs=stfm[:, :width], start=True, stop=True)
            sg = fpool.tile([64, GPB * 128], f32, tag="sg")
            nc.scalar.activation(sg[:, :width], zp[:, :width],
                                 mybir.ActivationFunctionType.Sigmoid)
            hf = fpool.tile([64, GPB * 128], f32, tag="hf")
            nc.sync.dma_start(
                hf[:, :width], ins["hfm"][:, gs * 128:gs * 128 + width])
            yt = fpool.tile([64, GPB * 128], f32, tag="yt")
            nc.vector.tensor_tensor(out=yt[:, :width], in0=sg[:, :width],
                                    in1=hf[:, :width], op=ADD)
            nc.sync.dma_start(
                outs["y"][:, gs * 128:gs * 128 + width], yt[:, :width])

        for hop in range(NUM_HOPS):
            for dirn in range(2):
                lay = lays[dirn]
                blocks = lay["blocks"]
                sched = lay["sched"]
                xsrc = (x0[dirn] if hop == 0
                        else xbuf[dirn][(hop - 1) % 2].ap())
                xv = xsrc.rearrange("(c n) d -> c n d", n=CHUNK)
                last = hop == NUM_HOPS - 1 and dirn == 1

                for bi, (g0, gc) in enumerate(blocks):
                    nb = lay["nblk"][bi]
                    neb = lay["neblk"][bi]
                    ob = lay["Oreal"][(bi, 0)]
                    oeb = lay["Oext"][(bi, 0)]
                    msgs = spool.tile([128, nsmax, D], f32r, tag="msgs")
                    for c in range(NCHUNKS):
                        ns = lay["nreal"][(bi, c)]
                        if ns == 0:
                            continue
                        o = lay["Oreal"][(bi, c)]
                        ol = o - ob
                        nc.gpsimd.dma_gather(
                            out_ap=msgs[:, ol:ol + ns, :],
                            in_ap=xv[c].bitcast(f32r),
                            idxs_ap=idx_res[dirn][:, o * 8:(o + ns) * 8],
                            num_idxs=ns * 128,
                            num_idxs_reg=ns * 128,
                            elem_size=D,
                            single_packet=False,
                            queue_num=c,
                        )
                    S = spool2.tile([128, nemax, 128], bf16, tag="S")
                    nc.vector.tensor_tensor(
                        out=S[:, :neb, :],
                        in0=iota_s[:].rearrange(
                            "p (o c) -> p o c", c=128).broadcast_to(
                            [128, neb, 128]),
                        in1=denc_res[dirn][:, oeb:oeb + neb].broadcast_to(
                            [128, neb, 128]),
                        op=EQ)
                    mbf = spool2.tile([128, nsmax, D], bf16, tag="mbf")
                    nc.vector.tensor_tensor(
                        out=mbf[:, :nb, :],
                        in0=msgs[:, :nb, :].bitcast(f32),
                        in1=val_res[dirn][:, ob:ob + nb].broadcast_to(
                            [128, nb, D]),
                        op=MUL)

                    for gl in range(gc):
                        g = g0 + gl
                        mms = sched[g]
                        if not mms:
                            continue
                        ps = pspool.tile([128, D], f32, tag="ps")
                        for j, (c, exl, rel) in enumerate(mms):
                            exb = lay["Oext"][(bi, c)] - oeb + exl
                            reb = lay["Oreal"][(bi, c)] - ob + rel
                            nc.tensor.matmul(
                                ps[:],
                                lhsT=S[:, exb, :],
                                rhs=mbf[:, reb, :],
                                start=(j == 0),
                                stop=(j == len(mms) - 1),
                            )
                        nc.vector.scalar_tensor_tensor(
                            out=st[:, g, :], in0=ps[:],
                            scalar=coef[hop][dirn], in1=st[:, g, :],
                            op0=MUL, op1=ADD)
                        if hop < NUM_HOPS - 1:
                            xc = xpool.tile([128, D], f32r, tag="xc")
                            nc.scalar.copy(out=xc[:], in_=ps[:])
                            bounce_v = bounce[dirn].ap().rearrange(
                                "(g p) f -> p g f", p=128)
                            nc.sync.dma_start(bounce_v[:, g, :], xc[:])
                    if last:
                        emit_final(g0, gc)

                if hop < NUM_HOPS - 1:
                    nc.gpsimd.collective_compute(
                        "AllGather", mybir.AluOpType.bypass,
                        replica_groups=rg,
                        ins=[bounce[dirn].ap().opt()],
                        outs=[xbuf[dirn][hop % 2].ap().opt()],
                    )


def kernel(**inputs) -> np.ndarray:
    return _run(inputs, trace=False)[0]


def kernel_traced(inputs, trace_kwargs=None):
    """Returns (output, BassKernelResults) with NTFF trace if available."""
    return _run(inputs, trace=True, trace_kwargs=trace_kwargs or {})


def _run(inputs, trace=False, trace_kwargs=None):
    import concourse.bacc as bacc
    import concourse.mybir as mybir
    import concourse.tile as tile
    from concourse.bass_utils import run_bass_kernel_spmd

    in_maps, meta = prep_host(**inputs)

    nc = bacc.Bacc("TRN2", target_bir_lowering=False, debug=False,
                   num_devices=NCORES, num_swdge_queues=4)
    f32 = mybir.dt.float32
    f32r = mybir.dt.float32r
    bf16 = mybir.dt.bfloat16
    i16 = mybir.dt.int16
    nsmax = meta["nsmax"]
    nemax = meta["nemax"]
    r0 = meta["lay"][0]["nreal_tot"]
    e0 = meta["lay"][0]["next_tot"]
    r1 = meta["lay"][1]["nreal_tot"]
    e1 = meta["lay"][1]["next_tot"]

    ins = {}
    shapes = {
        "x0_out": ([NODES_PAD, D], f32r),
        "x0_in": ([NODES_PAD, D], f32r),
        "hfm": ([D, SHARD], f32),
        "theta": ([D, D], bf16),
        "ident": ([128, 128], f32),
        "iota": ([128, 128], bf16),
        "idx0": ([128, r0 * 8], i16),
        "denc0": ([128, e0], bf16),
        "val0": ([128, r0], f32),
        "idx1": ([128, r1 * 8], i16),
        "denc1": ([128, e1], bf16),
        "val1": ([128, r1], f32),
    }
    for k, (shape, dt) in shapes.items():
        ins[k] = nc.dram_tensor(k, shape, dt, kind="ExternalInput").ap()
    y = nc.dram_tensor("y", [D, SHARD], f32, kind="ExternalOutput")

    with tile.TileContext(nc) as tc:
        build_program(tc, ins, {"y": y.ap()}, meta)
    nc.compile()

    kw = {}
    if trace:
        kw = dict(trace=True, trace_kwargs=trace_kwargs or {})
    res = run_bass_kernel_spmd(nc, in_maps, core_ids=list(range(NCORES)),
                               **kw)
    shards = [r["y"].T for r in res.results]  # each [SHARD, 64]
    out = np.concatenate(shards, axis=0)[:N_NODES]
    return np.ascontiguousarray(out.astype(np.float32)), res
